# revision 20
# baseline (speedup 1.0000x reference)
"""Trainium2 Bass kernel for DecoderWithTemporalAttention (single-step decode).

Math (reference collapses to, since initial decoder state is zero):
    re1    = tanh(h @ Ud_w.T + (Ud_b + Wd_b))          # [B, T, E]
    scores = re1 @ vd_w[0]                              # [B, T]  (+vd_b, dropped: softmax-invariant)
    beta   = softmax(scores, axis=T)
    c_t    = einsum('bt,bte->be', beta, h)              # [B, E]
    y_til  = concat([c_t, y[:,0]], -1) @ wt_w[0] + wt_b # [B]
    gates  = outer(y_til, W_ih[:,0]) + (b_ih + b_hh)    # [B, 4H]
    i,f,g,o = split(gates); d_new = sigmoid(o) * tanh(sigmoid(i) * tanh(g))
    returns stack([d_new, c_t])                         # [2, B, 256]

Sharding: pure data-parallel, batch 256 -> 8 cores x 32.

Device layout per core (BL=32 local batches):
  - hT  [BL,256,512] transposed on host, ei-major (moving operand for ud)
  - hn  [BL,512,257] natural rows + a 257th col h.wt (moving operand for c_t;
        the extra col folds the wt dot product into the same matmul)
  - ud matmul: stationary Ud_w.T tiles, out [e_out(128x2), (r,t)=1024] in PSUM
  - tanh+bias on ScalarE (per-partition bias, e_out on partitions) -> re1 bf16
  - scores: M=1 matmul vd-stationary -> PSUM [1,512] rows packed 4/bank at
    partitions {0,32,64,96}
  - softmax per group of 4 read directly from PSUM (max / exp+accum_out bf16)
  - unnormalized exp weights transposed via PE -> ct matmul; normalization is
    a single fused tensor_scalar_mul (x 1/zsum) at PSUM evacuation
  - LSTM-cell tail: K=1 outer-product matmul for gates, ACT sigmoid/tanh, DVE muls
"""

import os
import sys

import numpy as np

for _p in ("/opt/trn_rl_repo",):
    if _p not in sys.path and os.path.isdir(_p):
        sys.path.append(_p)

B, T, HE, HD = 256, 512, 256, 256
HE1 = HE + 1  # natural h row + folded wt column
NCORES = 8
BL = B // NCORES  # 32 local batches per core
G4 = BL // 4      # groups of 4 batches

# dtype knobs (accuracy/perf tradeoff)
H_NAT_BF16 = os.environ.get("KERN_H_BF16", "1") == "1"   # natural h (c_t path)
HT_BF16 = os.environ.get("KERN_HT_BF16", "1") == "1"     # transposed h (scores path)
RE1_BF16 = os.environ.get("KERN_RE1_BF16", "1") == "1"   # tanh output (scores matmul moving)

_cache = {}
DEBUG_GROUPS = int(os.environ.get("KERN_GROUPS", str(G4)))
SKIP_TAIL = os.environ.get("KERN_SKIP_TAIL", "0") == "1"
HP_BUFS = int(os.environ.get("KERN_HP_BUFS", "8"))


def _build_nc():
    from concourse import bacc, bass, mybir
    from concourse.tile import TileContext

    f32 = mybir.dt.float32
    bf16 = mybir.dt.bfloat16
    ht_dt = bf16 if HT_BF16 else f32
    h_dt = bf16 if H_NAT_BF16 else f32
    re1_dt = bf16 if RE1_BF16 else f32
    vd_dt = re1_dt  # scores-matmul stationary must not be lone-fp32
    AF = mybir.ActivationFunctionType

    nc = bacc.Bacc()

    htd_d = nc.declare_dram_parameter("htd", [BL // 2, 128, 2 * 2 * T], ht_dt, isOutput=False)
    hnd_d = nc.declare_dram_parameter("hnd", [BL // 2, 128, 2 * 4 * HE1], h_dt, isOutput=False)
    udt_d = nc.declare_dram_parameter("udt", [HE, HE], ht_dt, isOutput=False)
    vd2_d = nc.declare_dram_parameter("vd2", [128, 2], vd_dt, isOutput=False)
    wdb2_d = nc.declare_dram_parameter("wdb2", [128, 2], f32, isOutput=False)
    # gates-matmul moving blob: rows (W_ih col, b_ih+b_hh, W_ih col again);
    # with stationary rows (yt_attn, 1, c2) this folds both bias adds into PE
    wb3_d = nc.declare_dram_parameter("wb3", [3, 4 * HD], f32, isOutput=False)
    oc2_d = nc.declare_dram_parameter("oc2", [2, BL], f32, isOutput=False)
    ident_d = nc.declare_dram_parameter("ident", [128, 128], bf16, isOutput=False)
    outd_d = nc.declare_dram_parameter("out_d", [BL, HD], f32, isOutput=True)
    outc_d = nc.declare_dram_parameter("out_c", [BL, HE], f32, isOutput=True)

    with TileContext(nc) as tc:
        with (
            tc.tile_pool(name="const", bufs=1) as constp,
            tc.tile_pool(name="hp", bufs=HP_BUFS) as hp,
            tc.tile_pool(name="hnp", bufs=HP_BUFS) as hnp,
            tc.tile_pool(name="re1p", bufs=3) as re1p,
            tc.tile_pool(name="smp", bufs=3) as smp,
            # tiles read by a DMA (gather/out): long-recycle pool, so engine
            # ops rarely carry a WAR wait on a DMA semaphore
            tc.tile_pool(name="nrp", bufs=G4) as nrp,
            tc.tile_pool(name="tailp", bufs=1) as tailp,
            tc.tile_pool(name="tlp", bufs=2) as tlp,
            tc.tile_pool(name="udps", bufs=2, space="PSUM") as udps,
            tc.tile_pool(name="scps", bufs=1, space="PSUM") as scps,
            tc.tile_pool(name="trps", bufs=1, space="PSUM") as trps,
            tc.tile_pool(name="ctps", bufs=2, space="PSUM") as ctps,
        ):
            # ---- constants ----
            udt_sb = []
            for i in range(2):
                t_ = constp.tile([128, HE], ht_dt, tag=f"udt{i}")
                nc.sync.dma_start(out=t_[:], in_=udt_d[i * 128 : (i + 1) * 128, :])
                udt_sb.append(t_)
            vd2_sb = constp.tile([128, 2], vd_dt, tag="vd2")
            nc.sync.dma_start(out=vd2_sb[:], in_=vd2_d[:])
            wdb2_sb = constp.tile([128, 2], f32, tag="wdb2")
            nc.sync.dma_start(out=wdb2_sb[:], in_=wdb2_d[:])
            wb3_sb = constp.tile([3, 4 * HD], f32, tag="wb3")
            nc.sync.dma_start(out=wb3_sb[:], in_=wb3_d[:])
            # yt3: gates-matmul stationary; rows 1:3 (ones, c2) are constants,
            # row 0 is filled per tail block by the y_tilde gather DMA
            yt3 = tailp.tile([3, BL], f32, tag="yt3")
            nc.sync.dma_start(out=yt3[1:3, :], in_=oc2_d[:])
            ident_sb = constp.tile([128, 128], bf16, tag="ident")
            nc.sync.dma_start(out=ident_sb[:], in_=ident_d[:])

            # ---- per-engine warmups ----
            # Every engine observes each constant's DMA semaphore via a cheap
            # op up front; later ops then need <=1 sync wait (the hardware
            # instruction structs encode only one wait command).
            warm = ctps.tile([128, T], f32, tag="ct")
            for k, cst in enumerate(
                [udt_sb[0], udt_sb[1], vd2_sb, ident_sb, wb3_sb]
            ):
                nc.tensor.matmul(
                    warm[0:1, k : k + 1], cst[0:1, 0:1], cst[0:1, 0:1],
                    start=True, stop=True,
                )
            dscr = tailp.tile([1, 8], f32, tag="dscr")
            nc.vector.tensor_copy(dscr[0:1, 0:1], wb3_sb[0:1, 0:1])
            ascr = tailp.tile([1, 8], f32, tag="ascr")
            nc.scalar.copy(ascr[0:1, 0:1], wdb2_sb[0:1, 0:1])
            # chained: materialize the float-bias const AP on ACT only
            nc.scalar.activation(ascr[0:1, 1:2], ascr[0:1, 0:1], AF.Tanh, bias=0.0)

            ytacc = tailp.tile([128, G4], f32, tag="ytacc")

            def absorb(ps, mov=None):
                # tiny const matmul into a freshly allocated PSUM tile: takes
                # over the slot-release wait so the first real matmul into the
                # tile carries only its own (single) cross-engine wait
                nc.tensor.matmul(
                    ps[0:1, 0:1], vd2_sb[0:1, 0:1],
                    (mov if mov is not None else vd2_sb)[0:1, 0:1],
                    start=True, stop=True,
                )

            # ---- per-group-of-4 pipeline ----
            for g in range(DEBUG_GROUPS):
                h_pair = [None, None]
                sc = scps.tile([128, T], f32, tag="sc")
                absorb(sc)
                for qq in range(2):  # two pairs of batches in this group
                    q = 2 * g + qq
                    # load one pair (2 batches): hT first (ud needs it first)
                    hT_sb = hp.tile([128, 2 * 2 * T], ht_dt, tag="ht")
                    nc.sync.dma_start(out=hT_sb[:], in_=htd_d[q])
                    h_sb = hnp.tile([128, 2 * 4 * HE1], h_dt, tag="hn")
                    nc.sync.dma_start(out=h_sb[:], in_=hnd_d[q])
                    h_pair[qq] = h_sb
                    # ud matmuls for both batches of the pair (PSUM-bank limit:
                    # a matmul output may not exceed 512 f32 columns)
                    ud_eo = []
                    for eo in range(2):
                        ud = udps.tile([128, 2 * T], f32, tag="ud")
                        absorb(ud)
                        ud_eo.append(ud)
                        for r in range(2):
                            for ei in range(2):
                                nc.tensor.matmul(
                                    ud[:, r * T : (r + 1) * T],
                                    udt_sb[ei][:, eo * 128 : (eo + 1) * 128],
                                    hT_sb[:, (ei * 2 + r) * T : (ei * 2 + r + 1) * T],
                                    start=(ei == 0),
                                    stop=(ei == 1),
                                )
                    # tanh(+bias) over the pair -> re1 [128, (eo, pair-batch, t)]
                    re1 = re1p.tile([128, 2 * 2 * T], re1_dt, tag="re1")
                    for eo in range(2):
                        nc.scalar.activation(
                            re1[:, eo * 2 * T : (eo + 1) * 2 * T],
                            ud_eo[eo][:],
                            AF.Tanh,
                            bias=wdb2_sb[:, eo : eo + 1],
                        )
                    # scores for both batches -> psum rows {0,32,64,96}
                    for r in range(2):
                        jj = 2 * qq + r
                        for ei in range(2):
                            nc.tensor.matmul(
                                sc[32 * jj : 32 * jj + 32, :],
                                vd2_sb[:, ei : ei + 1].broadcast_to([128, 32]),
                                re1[:, (ei * 2 + r) * T : (ei * 2 + r + 1) * T],
                                start=(ei == 0),
                                stop=(ei == 1),
                                tile_position=(0, 32 * jj),
                            )

                # ---- softmax over this group of 4, read directly from PSUM --
                # score rows live at partitions {0,32,64,96}; every partition p
                # holds a copy of batch p//32's scores, so full-128 ops work.
                mx = smp.tile([128, 1], f32, tag="mx")
                nc.vector.reduce_max(mx[:], sc[:], axis=mybir.AxisListType.X)
                nmx = smp.tile([128, 1], f32, tag="nmx")
                nc.vector.tensor_scalar_mul(nmx[:], mx[:], -1.0)
                pexp = nrp.tile([128, T], re1_dt, tag="pexp")
                zsum = smp.tile([128, 1], f32, tag="zsum")
                nc.scalar.activation(
                    pexp[:], sc[:], AF.Exp, bias=nmx[:], accum_out=zsum[:]
                )
                rz = smp.tile([128, 1], f32, tag="rz")
                nc.vector.reciprocal(rz[:], zsum[:])
                # gather the 4 (unnormalized) weight rows into contiguous
                # partitions (DMA moves freely across partitions; engines can't)
                beta = smp.tile([4, T], re1_dt, tag="beta")
                # DVE dummy write absorbs the slot-release wait so the gather
                # DMA carries only its producer wait
                nc.vector.tensor_copy(beta[0:1, 0:1], dscr[0:1, 0:1])
                nc.gpsimd.dma_start(
                    out=beta[:],
                    in_=pexp[:].rearrange("(j s) t -> j s t", s=32)[:, 0, :],
                )

                # ---- transpose weights -> betaT columns [t=128 x 4tt, batch] --
                btr = trps.tile([128, 32], re1_dt, tag="btr")
                nc.tensor.transpose(
                    btr[0:1, 0:1], ident_sb[0:1, 0:1], ident_sb[0:1, 0:1]
                )
                for tt in range(4):
                    nc.tensor.transpose(
                        btr[:, tt * 4 : (tt + 1) * 4],
                        beta[:, tt * 128 : (tt + 1) * 128],
                        ident_sb[0:4, 0:4],
                    )
                betaT = smp.tile([128, 16], h_dt, tag="betaT")
                nc.vector.tensor_copy(betaT[:], btr[:, 0:16])

                # ---- c_t (+ folded wt column) for the 4 batches ----
                ct = ctps.tile([128, T], f32, tag="ct")
                absorb(ct, mov=h_pair[0])
                absorb(ct, mov=h_pair[1])
                for jj in range(4):
                    qq, r = divmod(jj, 2)
                    for tt in range(4):
                        nc.tensor.matmul(
                            ct[32 * jj : 32 * jj + 32, 0:HE1],
                            betaT[:, tt * 4 + jj : tt * 4 + jj + 1].broadcast_to(
                                [128, 32]
                            ),
                            h_pair[qq][:, (r * 4 + tt) * HE1 : (r * 4 + tt + 1) * HE1],
                            start=(tt == 0),
                            stop=(tt == 3),
                            tile_position=(0, 32 * jj),
                        )
                # single fused evacuation: normalize by 1/zsum on the way out
                ctstage = nrp.tile([128, HE1], f32, tag="ctstage")
                nc.vector.tensor_scalar_mul(ctstage[:], ct[:, 0:HE1], rz[:])
                # c_t output rows for this group (DMA un-strides the rows)
                nc.gpsimd.dma_start(
                    out=outc_d[4 * g : 4 * g + 4, :],
                    in_=ctstage[:].rearrange("(j s) e -> j s e", s=32)[:, 0, 0:HE],
                )
                # wt . c_t partial of y_tilde came out in the folded column
                nc.vector.tensor_copy(ytacc[:, g : g + 1], ctstage[:, HE:HE1])

                # ---- tail block every 2 groups: y_tilde, gates, LSTM cell ----
                # gates = yt_attn*W_ih + 1*(b_ih+b_hh) + c2*W_ih via the K=3
                # stationary (yt3) against the wb3 moving blob; f gate unused
                # since c0 = 0. Runs overlapped with later groups' main work.
                if SKIP_TAIL or (g % 2 == 0):
                    continue
                k = g // 2
                # y_tilde-attn entries for groups 2k,2k+1 -> yt3 row 0, cols
                # [8k:8k+8]; stream order p = j*2+gg holds batch 8k+4*gg+j
                nc.gpsimd.dma_start(
                    out=yt3[0:1, 8 * k : 8 * k + 8],
                    in_=ytacc[:].rearrange("(j s) g -> j s g", s=32)[
                        :, 0, 2 * k : 2 * k + 2
                    ],
                )
                gps = []
                for half in range(2):
                    g_ = ctps.tile([128, T], f32, tag="ct")
                    absorb(g_)
                    nc.tensor.matmul(
                        g_[0:8, 0:512],
                        yt3[0:3, 8 * k : 8 * k + 8],
                        wb3_sb[:, half * 512 : (half + 1) * 512],
                    )
                    gps.append(g_)
                # gates: i=[0:256], g=[512:768], o=[768:1024] read from PSUM
                gi = tlp.tile([8, HD], f32, tag="gi")
                nc.scalar.activation(gi[:], gps[0][0:8, 0:256], AF.Sigmoid)
                gg = tlp.tile([8, HD], f32, tag="gg")
                nc.scalar.activation(gg[:], gps[1][0:8, 0:256], AF.Tanh)
                go = tlp.tile([8, HD], f32, tag="go")
                nc.scalar.activation(go[:], gps[1][0:8, 256:512], AF.Sigmoid)
                cnew = tlp.tile([8, HD], f32, tag="cnew")
                nc.vector.tensor_mul(cnew[:], gi[:], gg[:])
                tcn = tlp.tile([8, HD], f32, tag="tcn")
                nc.scalar.activation(tcn[:], cnew[:], AF.Tanh)
                dnew = nrp.tile([8, HD], f32, tag="dnew")
                nc.vector.tensor_mul(dnew[:], go[:], tcn[:])
                # dnew partition p = j*2+gg is batch 8k + 4*gg + j
                nc.gpsimd.dma_start(
                    out=outd_d[8 * k : 8 * k + 8].rearrange(
                        "(gg j) e -> j gg e", j=4
                    ),
                    in_=dnew[:],
                )

    nc.compile()
    return nc


def _prep_in_maps(inputs):
    h = np.asarray(inputs["h_t_enc"], np.float32)
    y = np.asarray(inputs["y"], np.float32)
    Ud_w = np.asarray(inputs["Ud_w"], np.float32)
    Ud_b = np.asarray(inputs["Ud_b"], np.float32)
    Wd_b = np.asarray(inputs["Wd_b"], np.float32)
    vd_w = np.asarray(inputs["vd_w"], np.float32)
    wt_w = np.asarray(inputs["wt_w"], np.float32)
    wt_b = np.asarray(inputs["wt_b"], np.float32)
    W_ih = np.asarray(inputs["W_ih"], np.float32)
    b_ih = np.asarray(inputs["b_ih"], np.float32)
    b_hh = np.asarray(inputs["b_hh"], np.float32)

    from ml_dtypes import bfloat16

    h_dt = bfloat16 if H_NAT_BF16 else np.float32
    ht_dt = bfloat16 if HT_BF16 else np.float32
    vd_dt = bfloat16 if RE1_BF16 else np.float32

    udt = np.ascontiguousarray(Ud_w.T).astype(ht_dt)
    vd2 = np.ascontiguousarray(vd_w[0].reshape(2, 128).T).astype(vd_dt)
    wdb2 = np.ascontiguousarray((Wd_b + Ud_b).reshape(2, 128).T)
    wb3 = np.ascontiguousarray(
        np.stack([W_ih[:, 0], b_ih + b_hh, W_ih[:, 0]], axis=0)
    )
    ident = np.eye(128, dtype=np.float32).astype(bfloat16)

    wt_vec = wt_w[0, :HE]

    def make_ht(hc):
        # hT region, ei-major: [pair, p, et(2), rb(2), t(512)]
        return np.ascontiguousarray(
            hc.transpose(0, 2, 1).reshape(BL // 2, 2, 2, 128, T)
            .transpose(0, 3, 2, 1, 4).reshape(BL // 2, 128, 2 * 2 * T)
        ).astype(ht_dt)

    def make_hn(hc):
        # natural rows + folded wt col: [pair, p, rb(2), tt(4), e(257)]
        hw = hc @ wt_vec  # [BL, T]
        nat = np.empty((BL // 2, 2, 4, 128, HE1), np.float32)
        nat[..., :HE] = hc.reshape(BL // 2, 2, 4, 128, HE)
        nat[..., HE] = hw.reshape(BL // 2, 2, 4, 128)
        return np.ascontiguousarray(
            nat.transpose(0, 3, 1, 2, 4).reshape(BL // 2, 128, 2 * 4 * HE1)
        ).astype(h_dt)

    in_maps = []
    for c in range(NCORES):
        sl = slice(c * BL, (c + 1) * BL)
        hc = h[sl]
        # per-batch constant part of y_tilde, in tail-block stream order:
        # oc2[1, 8k + 2j + gg] = c2v[8k + 4gg + j]; row 0 is the ones row
        c2v = wt_w[0, HE] * y[sl, 0, 0] + wt_b[0]  # [BL]
        oc2 = np.ones((2, BL), np.float32)
        for k in range(BL // 8):
            for j in range(4):
                for gg in range(2):
                    oc2[1, 8 * k + 2 * j + gg] = c2v[8 * k + 4 * gg + j]
        in_maps.append(
            {
                "htd": make_ht(hc),
                "hnd": make_hn(hc),
                "udt": udt,
                "vd2": vd2,
                "wdb2": wdb2,
                "wb3": wb3,
                "oc2": oc2,
                "ident": ident,
            }
        )
    return in_maps


def kernel(**inputs):
    from concourse.bass_utils import run_bass_kernel_spmd

    key = (H_NAT_BF16, HT_BF16, RE1_BF16)
    if key not in _cache:
        _cache[key] = _build_nc()
    nc = _cache[key]

    in_maps = _prep_in_maps(inputs)
    res = run_bass_kernel_spmd(nc, in_maps, list(range(NCORES)))
    kernel.last_results = res

    d_new = np.concatenate([np.asarray(r["out_d"]) for r in res.results], axis=0)
    c_t = np.concatenate([np.asarray(r["out_c"]) for r in res.results], axis=0)
    return np.stack([d_new.astype(np.float32), c_t.astype(np.float32)], axis=0)


kernel.last_results = None


# revision 24
# speedup vs baseline: 1.1456x; 1.1456x over previous
"""Trainium2 Bass kernel for DecoderWithTemporalAttention (single-step decode).

Math (reference collapses to, since initial decoder state is zero):
    re1    = tanh(h @ Ud_w.T + (Ud_b + Wd_b))          # [B, T, E]
    scores = re1 @ vd_w[0]                              # [B, T]  (+vd_b, dropped: softmax-invariant)
    beta   = softmax(scores, axis=T)
    c_t    = einsum('bt,bte->be', beta, h)              # [B, E]
    y_til  = concat([c_t, y[:,0]], -1) @ wt_w[0] + wt_b # [B]
    gates  = outer(y_til, W_ih[:,0]) + (b_ih + b_hh)    # [B, 4H]
    i,f,g,o = split(gates); d_new = sigmoid(o) * tanh(sigmoid(i) * tanh(g))
    returns stack([d_new, c_t])                         # [2, B, 256]

Sharding: pure data-parallel, batch 256 -> 8 cores x 32.

Device layout per core (BL=32 local batches):
  - hT  [BL,256,512] transposed on host, ei-major (moving operand for ud)
  - hn  [BL,512,257] natural rows + a 257th col h.wt (moving operand for c_t;
        the extra col folds the wt dot product into the same matmul)
  - ud matmul: stationary Ud_w.T tiles, out [e_out(128x2), (r,t)=1024] in PSUM
  - tanh+bias on ScalarE (per-partition bias, e_out on partitions) -> re1 bf16
  - scores: M=1 matmul vd-stationary -> PSUM [1,512] rows packed 4/bank at
    partitions {0,32,64,96}
  - softmax per group of 4 read directly from PSUM (max / exp+accum_out bf16)
  - unnormalized exp weights transposed via PE -> ct matmul; normalization is
    a single fused tensor_scalar_mul (x 1/zsum) at PSUM evacuation
  - LSTM-cell tail: K=1 outer-product matmul for gates, ACT sigmoid/tanh, DVE muls
"""

import os
import sys

import numpy as np

for _p in ("/opt/trn_rl_repo",):
    if _p not in sys.path and os.path.isdir(_p):
        sys.path.append(_p)

B, T, HE, HD = 256, 512, 256, 256
HE1 = HE + 1  # natural h row + folded wt column
NCORES = 8
BL = B // NCORES  # 32 local batches per core
G4 = BL // 4      # groups of 4 batches

# dtype knobs (accuracy/perf tradeoff)
H_NAT_BF16 = os.environ.get("KERN_H_BF16", "1") == "1"   # natural h (c_t path)
HT_BF16 = os.environ.get("KERN_HT_BF16", "1") == "1"     # transposed h (scores path)
RE1_BF16 = os.environ.get("KERN_RE1_BF16", "1") == "1"   # tanh output (scores matmul moving)

_cache = {}
DEBUG_GROUPS = int(os.environ.get("KERN_GROUPS", str(G4)))
SKIP_TAIL = os.environ.get("KERN_SKIP_TAIL", "0") == "1"
HP_BUFS = int(os.environ.get("KERN_HP_BUFS", "8"))


def _build_nc():
    from concourse import bacc, bass, mybir
    from concourse.tile import TileContext

    f32 = mybir.dt.float32
    bf16 = mybir.dt.bfloat16
    ht_dt = bf16 if HT_BF16 else f32
    h_dt = bf16 if H_NAT_BF16 else f32
    re1_dt = bf16 if RE1_BF16 else f32
    vd_dt = re1_dt  # scores-matmul stationary must not be lone-fp32
    AF = mybir.ActivationFunctionType

    nc = bacc.Bacc()

    htd_d = nc.declare_dram_parameter("htd", [BL // 2, 128, 2 * 2 * T], ht_dt, isOutput=False)
    hnd_d = nc.declare_dram_parameter("hnd", [BL // 2, 128, 2 * 4 * HE1], h_dt, isOutput=False)
    udt_d = nc.declare_dram_parameter("udt", [HE, HE], ht_dt, isOutput=False)
    vd2_d = nc.declare_dram_parameter("vd2", [128, 2], vd_dt, isOutput=False)
    wdb2_d = nc.declare_dram_parameter("wdb2", [128, 2], f32, isOutput=False)
    # gates-matmul moving blob: rows (W_ih col, b_ih+b_hh, W_ih col again);
    # with stationary rows (yt_attn, 1, c2) this folds both bias adds into PE
    wb3_d = nc.declare_dram_parameter("wb3", [3, 4 * HD], f32, isOutput=False)
    oc2_d = nc.declare_dram_parameter("oc2", [2, BL], f32, isOutput=False)
    ident_d = nc.declare_dram_parameter("ident", [128, 128], bf16, isOutput=False)
    outd_d = nc.declare_dram_parameter("out_d", [BL, HD], f32, isOutput=True)
    outc_d = nc.declare_dram_parameter("out_c", [BL, HE], f32, isOutput=True)

    with TileContext(nc) as tc:
        with (
            tc.tile_pool(name="const", bufs=1) as constp,
            tc.tile_pool(name="hp", bufs=HP_BUFS) as hp,
            tc.tile_pool(name="hnp", bufs=HP_BUFS) as hnp,
            tc.tile_pool(name="re1p", bufs=3) as re1p,
            tc.tile_pool(name="smp", bufs=3) as smp,
            # tiles read by a DMA (gather/out): long-recycle pool, so engine
            # ops rarely carry a WAR wait on a DMA semaphore
            tc.tile_pool(name="nrp", bufs=G4) as nrp,
            tc.tile_pool(name="tailp", bufs=1) as tailp,
            tc.tile_pool(name="tlp", bufs=2) as tlp,
            tc.tile_pool(name="udps", bufs=2, space="PSUM") as udps,
            tc.tile_pool(name="scps", bufs=1, space="PSUM") as scps,
            tc.tile_pool(name="trps", bufs=1, space="PSUM") as trps,
            tc.tile_pool(name="ctps", bufs=2, space="PSUM") as ctps,
        ):
            # ---- constants ----
            udt_sb = []
            for i in range(2):
                t_ = constp.tile([128, HE], ht_dt, tag=f"udt{i}")
                nc.sync.dma_start(out=t_[:], in_=udt_d[i * 128 : (i + 1) * 128, :])
                udt_sb.append(t_)
            vd2_sb = constp.tile([128, 2], vd_dt, tag="vd2")
            nc.sync.dma_start(out=vd2_sb[:], in_=vd2_d[:])
            wdb2_sb = constp.tile([128, 2], f32, tag="wdb2")
            nc.sync.dma_start(out=wdb2_sb[:], in_=wdb2_d[:])
            wb3_sb = constp.tile([3, 4 * HD], f32, tag="wb3")
            nc.sync.dma_start(out=wb3_sb[:], in_=wb3_d[:])
            # yt3: gates-matmul stationary; rows 1:3 (ones, c2) are constants,
            # row 0 is filled per tail block by the y_tilde gather DMA
            yt3 = tailp.tile([3, BL], f32, tag="yt3")
            nc.sync.dma_start(out=yt3[1:3, :], in_=oc2_d[:])
            ident_sb = constp.tile([128, 128], bf16, tag="ident")
            nc.sync.dma_start(out=ident_sb[:], in_=ident_d[:])

            # ---- per-engine warmups ----
            # Every engine observes each constant's DMA semaphore via a cheap
            # op up front; later ops then need <=1 sync wait (the hardware
            # instruction structs encode only one wait command).
            warm = ctps.tile([128, T], f32, tag="ct")
            for k, cst in enumerate(
                [udt_sb[0], udt_sb[1], vd2_sb, ident_sb, wb3_sb]
            ):
                nc.tensor.matmul(
                    warm[0:1, k : k + 1], cst[0:1, 0:1], cst[0:1, 0:1],
                    start=True, stop=True,
                )
            dscr = tailp.tile([1, 8], f32, tag="dscr")
            nc.vector.tensor_copy(dscr[0:1, 0:1], wb3_sb[0:1, 0:1])
            ascr = tailp.tile([1, 8], f32, tag="ascr")
            nc.scalar.copy(ascr[0:1, 0:1], wdb2_sb[0:1, 0:1])
            # chained: materialize the float-bias const AP on ACT only
            nc.scalar.activation(ascr[0:1, 1:2], ascr[0:1, 0:1], AF.Tanh, bias=0.0)

            ytacc = tailp.tile([128, G4], f32, tag="ytacc")

            def emit_tail(k):
                # ---- tail block for groups 2k,2k+1: gates + LSTM cell ----
                # gates = yt_attn*W_ih + 1*(b_ih+b_hh) + c2*W_ih via the K=3
                # stationary (yt3) against the wb3 moving blob; f gate unused
                # since c0 = 0. Emitted two groups after its gather so the PE
                # stream never stalls on the evac->gather chain.
                gps = []
                for half in range(2):
                    g_ = ctps.tile([128, T], f32, tag="ct")
                    absorb(g_)
                    nc.tensor.matmul(
                        g_[0:8, 0:512],
                        yt3[0:3, 8 * k : 8 * k + 8],
                        wb3_sb[:, half * 512 : (half + 1) * 512],
                    )
                    gps.append(g_)
                # gates: i=[0:256], g=[512:768], o=[768:1024] read from PSUM
                gi = tlp.tile([8, HD], f32, tag="gi")
                nc.scalar.activation(gi[:], gps[0][0:8, 0:256], AF.Sigmoid)
                gg = tlp.tile([8, HD], f32, tag="gg")
                nc.scalar.activation(gg[:], gps[1][0:8, 0:256], AF.Tanh)
                go = tlp.tile([8, HD], f32, tag="go")
                nc.scalar.activation(go[:], gps[1][0:8, 256:512], AF.Sigmoid)
                cnew = tlp.tile([8, HD], f32, tag="cnew")
                nc.vector.tensor_mul(cnew[:], gi[:], gg[:])
                tcn = tlp.tile([8, HD], f32, tag="tcn")
                nc.scalar.activation(tcn[:], cnew[:], AF.Tanh)
                dnew = nrp.tile([8, HD], f32, tag="dnew")
                nc.vector.tensor_mul(dnew[:], go[:], tcn[:])
                # dnew partition p = j*2+gg is batch 8k + 4*gg + j
                nc.sync.dma_start(
                    out=outd_d[8 * k : 8 * k + 8].rearrange(
                        "(gg j) e -> j gg e", j=4
                    ),
                    in_=dnew[:],
                )

            def absorb(ps, mov=None):
                # tiny const matmul into a freshly allocated PSUM tile: takes
                # over the slot-release wait so the first real matmul into the
                # tile carries only its own (single) cross-engine wait
                nc.tensor.matmul(
                    ps[0:1, 0:1], vd2_sb[0:1, 0:1],
                    (mov if mov is not None else vd2_sb)[0:1, 0:1],
                    start=True, stop=True,
                )

            # ---- per-group-of-4 pipeline ----
            tails_emitted = 0
            for g in range(DEBUG_GROUPS):
                # deferred tail blocks: block k's gather was issued at the end
                # of group 2k+1; emit its compute at the top of group 2k+3
                if not SKIP_TAIL and g % 2 == 1 and g >= 3:
                    emit_tail(tails_emitted)
                    tails_emitted += 1
                h_pair = [None, None]
                sc = scps.tile([128, T], f32, tag="sc")
                absorb(sc)
                for qq in range(2):  # two pairs of batches in this group
                    q = 2 * g + qq
                    # load one pair (2 batches): hT first (ud needs it first)
                    hT_sb = hp.tile([128, 2 * 2 * T], ht_dt, tag="ht")
                    nc.sync.dma_start(out=hT_sb[:], in_=htd_d[q])
                    h_sb = hnp.tile([128, 2 * 4 * HE1], h_dt, tag="hn")
                    nc.sync.dma_start(out=h_sb[:], in_=hnd_d[q])
                    h_pair[qq] = h_sb
                    # ud matmuls for both batches of the pair (PSUM-bank limit:
                    # a matmul output may not exceed 512 f32 columns)
                    ud_eo = []
                    for eo in range(2):
                        ud = udps.tile([128, 2 * T], f32, tag="ud")
                        absorb(ud)
                        ud_eo.append(ud)
                        for r in range(2):
                            for ei in range(2):
                                nc.tensor.matmul(
                                    ud[:, r * T : (r + 1) * T],
                                    udt_sb[ei][:, eo * 128 : (eo + 1) * 128],
                                    hT_sb[:, (ei * 2 + r) * T : (ei * 2 + r + 1) * T],
                                    start=(ei == 0),
                                    stop=(ei == 1),
                                )
                    # tanh(+bias) over the pair -> re1 [128, (eo, pair-batch, t)]
                    re1 = re1p.tile([128, 2 * 2 * T], re1_dt, tag="re1")
                    for eo in range(2):
                        nc.scalar.activation(
                            re1[:, eo * 2 * T : (eo + 1) * 2 * T],
                            ud_eo[eo][:],
                            AF.Tanh,
                            bias=wdb2_sb[:, eo : eo + 1],
                        )
                    # scores for both batches -> psum rows {0,32,64,96}
                    for r in range(2):
                        jj = 2 * qq + r
                        for ei in range(2):
                            nc.tensor.matmul(
                                sc[32 * jj : 32 * jj + 32, :],
                                vd2_sb[:, ei : ei + 1].broadcast_to([128, 32]),
                                re1[:, (ei * 2 + r) * T : (ei * 2 + r + 1) * T],
                                start=(ei == 0),
                                stop=(ei == 1),
                                tile_position=(0, 32 * jj),
                            )

                # ---- softmax over this group of 4, read directly from PSUM --
                # score rows live at partitions {0,32,64,96}; every partition p
                # holds a copy of batch p//32's scores, so full-128 ops work.
                mx = smp.tile([128, 1], f32, tag="mx")
                nc.vector.reduce_max(mx[:], sc[:], axis=mybir.AxisListType.X)
                nmx = smp.tile([128, 1], f32, tag="nmx")
                nc.vector.tensor_scalar_mul(nmx[:], mx[:], -1.0)
                pexp = nrp.tile([128, T], re1_dt, tag="pexp")
                zsum = smp.tile([128, 1], f32, tag="zsum")
                nc.scalar.activation(
                    pexp[:], sc[:], AF.Exp, bias=nmx[:], accum_out=zsum[:]
                )
                rz = smp.tile([128, 1], f32, tag="rz")
                nc.vector.reciprocal(rz[:], zsum[:])
                # gather the 4 (unnormalized) weight rows into contiguous
                # partitions (DMA moves freely across partitions; engines can't)
                beta = smp.tile([4, T], re1_dt, tag="beta")
                # DVE dummy write absorbs the slot-release wait so the gather
                # DMA carries only its producer wait
                nc.vector.tensor_copy(beta[0:1, 0:1], dscr[0:1, 0:1])
                nc.sync.dma_start(
                    out=beta[:],
                    in_=pexp[:].rearrange("(j s) t -> j s t", s=32)[:, 0, :],
                )

                # ---- transpose weights -> betaT columns [t=128 x 4tt, batch] --
                btr = trps.tile([128, 32], re1_dt, tag="btr")
                nc.tensor.transpose(
                    btr[0:1, 0:1], ident_sb[0:1, 0:1], ident_sb[0:1, 0:1]
                )
                for tt in range(4):
                    nc.tensor.transpose(
                        btr[:, tt * 4 : (tt + 1) * 4],
                        beta[:, tt * 128 : (tt + 1) * 128],
                        ident_sb[0:4, 0:4],
                    )
                betaT = smp.tile([128, 16], h_dt, tag="betaT")
                nc.vector.tensor_copy(betaT[:], btr[:, 0:16])

                # ---- c_t (+ folded wt column) for the 4 batches ----
                ct = ctps.tile([128, T], f32, tag="ct")
                absorb(ct, mov=h_pair[0])
                absorb(ct, mov=h_pair[1])
                for jj in range(4):
                    qq, r = divmod(jj, 2)
                    for tt in range(4):
                        nc.tensor.matmul(
                            ct[32 * jj : 32 * jj + 32, 0:HE1],
                            betaT[:, tt * 4 + jj : tt * 4 + jj + 1].broadcast_to(
                                [128, 32]
                            ),
                            h_pair[qq][:, (r * 4 + tt) * HE1 : (r * 4 + tt + 1) * HE1],
                            start=(tt == 0),
                            stop=(tt == 3),
                            tile_position=(0, 32 * jj),
                        )
                # single fused evacuation: normalize by 1/zsum on the way out
                ctstage = nrp.tile([128, HE1], f32, tag="ctstage")
                nc.vector.tensor_scalar_mul(ctstage[:], ct[:, 0:HE1], rz[:])
                # c_t output rows for this group (DMA un-strides the rows)
                nc.sync.dma_start(
                    out=outc_d[4 * g : 4 * g + 4, :],
                    in_=ctstage[:].rearrange("(j s) e -> j s e", s=32)[:, 0, 0:HE],
                )
                # wt . c_t partial of y_tilde came out in the folded column
                nc.vector.tensor_copy(ytacc[:, g : g + 1], ctstage[:, HE:HE1])

                # issue the y_tilde gather for this block of 2 groups; the
                # dependent compute is emitted two groups later (see top of
                # loop) so the gather latency is fully hidden
                if SKIP_TAIL or (g % 2 == 0):
                    continue
                k = g // 2
                # y_tilde-attn entries for groups 2k,2k+1 -> yt3 row 0, cols
                # [8k:8k+8]; stream order p = j*2+gg holds batch 8k+4*gg+j
                nc.sync.dma_start(
                    out=yt3[0:1, 8 * k : 8 * k + 8],
                    in_=ytacc[:].rearrange("(j s) g -> j s g", s=32)[
                        :, 0, 2 * k : 2 * k + 2
                    ],
                )

            if not SKIP_TAIL:
                for k in range(tails_emitted, DEBUG_GROUPS // 2):
                    emit_tail(k)

    nc.compile()
    return nc


def _prep_in_maps(inputs):
    h = np.asarray(inputs["h_t_enc"], np.float32)
    y = np.asarray(inputs["y"], np.float32)
    Ud_w = np.asarray(inputs["Ud_w"], np.float32)
    Ud_b = np.asarray(inputs["Ud_b"], np.float32)
    Wd_b = np.asarray(inputs["Wd_b"], np.float32)
    vd_w = np.asarray(inputs["vd_w"], np.float32)
    wt_w = np.asarray(inputs["wt_w"], np.float32)
    wt_b = np.asarray(inputs["wt_b"], np.float32)
    W_ih = np.asarray(inputs["W_ih"], np.float32)
    b_ih = np.asarray(inputs["b_ih"], np.float32)
    b_hh = np.asarray(inputs["b_hh"], np.float32)

    from ml_dtypes import bfloat16

    h_dt = bfloat16 if H_NAT_BF16 else np.float32
    ht_dt = bfloat16 if HT_BF16 else np.float32
    vd_dt = bfloat16 if RE1_BF16 else np.float32

    udt = np.ascontiguousarray(Ud_w.T).astype(ht_dt)
    vd2 = np.ascontiguousarray(vd_w[0].reshape(2, 128).T).astype(vd_dt)
    wdb2 = np.ascontiguousarray((Wd_b + Ud_b).reshape(2, 128).T)
    wb3 = np.ascontiguousarray(
        np.stack([W_ih[:, 0], b_ih + b_hh, W_ih[:, 0]], axis=0)
    )
    ident = np.eye(128, dtype=np.float32).astype(bfloat16)

    wt_vec = wt_w[0, :HE]

    def make_ht(hc):
        # hT region, ei-major: [pair, p, et(2), rb(2), t(512)]
        return np.ascontiguousarray(
            hc.transpose(0, 2, 1).reshape(BL // 2, 2, 2, 128, T)
            .transpose(0, 3, 2, 1, 4).reshape(BL // 2, 128, 2 * 2 * T)
        ).astype(ht_dt)

    def make_hn(hc):
        # natural rows + folded wt col: [pair, p, rb(2), tt(4), e(257)]
        hw = hc @ wt_vec  # [BL, T]
        nat = np.empty((BL // 2, 2, 4, 128, HE1), np.float32)
        nat[..., :HE] = hc.reshape(BL // 2, 2, 4, 128, HE)
        nat[..., HE] = hw.reshape(BL // 2, 2, 4, 128)
        return np.ascontiguousarray(
            nat.transpose(0, 3, 1, 2, 4).reshape(BL // 2, 128, 2 * 4 * HE1)
        ).astype(h_dt)

    in_maps = []
    for c in range(NCORES):
        sl = slice(c * BL, (c + 1) * BL)
        hc = h[sl]
        # per-batch constant part of y_tilde, in tail-block stream order:
        # oc2[1, 8k + 2j + gg] = c2v[8k + 4gg + j]; row 0 is the ones row
        c2v = wt_w[0, HE] * y[sl, 0, 0] + wt_b[0]  # [BL]
        oc2 = np.ones((2, BL), np.float32)
        for k in range(BL // 8):
            for j in range(4):
                for gg in range(2):
                    oc2[1, 8 * k + 2 * j + gg] = c2v[8 * k + 4 * gg + j]
        in_maps.append(
            {
                "htd": make_ht(hc),
                "hnd": make_hn(hc),
                "udt": udt,
                "vd2": vd2,
                "wdb2": wdb2,
                "wb3": wb3,
                "oc2": oc2,
                "ident": ident,
            }
        )
    return in_maps


def kernel(**inputs):
    from concourse.bass_utils import run_bass_kernel_spmd

    key = (H_NAT_BF16, HT_BF16, RE1_BF16)
    if key not in _cache:
        _cache[key] = _build_nc()
    nc = _cache[key]

    in_maps = _prep_in_maps(inputs)
    res = run_bass_kernel_spmd(nc, in_maps, list(range(NCORES)))
    kernel.last_results = res

    d_new = np.concatenate([np.asarray(r["out_d"]) for r in res.results], axis=0)
    c_t = np.concatenate([np.asarray(r["out_c"]) for r in res.results], axis=0)
    return np.stack([d_new.astype(np.float32), c_t.astype(np.float32)], axis=0)


kernel.last_results = None


# revision 25
# speedup vs baseline: 1.2150x; 1.0605x over previous
"""Trainium2 Bass kernel for DecoderWithTemporalAttention (single-step decode).

Math (reference collapses to, since initial decoder state is zero):
    re1    = tanh(h @ Ud_w.T + (Ud_b + Wd_b))          # [B, T, E]
    scores = re1 @ vd_w[0]                              # [B, T]  (+vd_b, dropped: softmax-invariant)
    beta   = softmax(scores, axis=T)
    c_t    = einsum('bt,bte->be', beta, h)              # [B, E]
    y_til  = concat([c_t, y[:,0]], -1) @ wt_w[0] + wt_b # [B]
    gates  = outer(y_til, W_ih[:,0]) + (b_ih + b_hh)    # [B, 4H]
    i,f,g,o = split(gates); d_new = sigmoid(o) * tanh(sigmoid(i) * tanh(g))
    returns stack([d_new, c_t])                         # [2, B, 256]

Sharding: pure data-parallel, batch 256 -> 8 cores x 32.

Device layout per core (BL=32 local batches):
  - hT  [BL,256,512] transposed on host, ei-major (moving operand for ud)
  - hn  [BL,512,257] natural rows + a 257th col h.wt (moving operand for c_t;
        the extra col folds the wt dot product into the same matmul)
  - ud matmul: stationary Ud_w.T tiles, out [e_out(128x2), (r,t)=1024] in PSUM
  - tanh+bias on ScalarE (per-partition bias, e_out on partitions) -> re1 bf16
  - scores: M=1 matmul vd-stationary -> PSUM [1,512] rows packed 4/bank at
    partitions {0,32,64,96}
  - softmax per group of 4 read directly from PSUM (max / exp+accum_out bf16)
  - unnormalized exp weights transposed via PE -> ct matmul; normalization is
    a single fused tensor_scalar_mul (x 1/zsum) at PSUM evacuation
  - LSTM-cell tail: K=1 outer-product matmul for gates, ACT sigmoid/tanh, DVE muls
"""

import os
import sys

import numpy as np

for _p in ("/opt/trn_rl_repo",):
    if _p not in sys.path and os.path.isdir(_p):
        sys.path.append(_p)

B, T, HE, HD = 256, 512, 256, 256
HE1 = HE + 1  # natural h row + folded wt column
NCORES = 8
BL = B // NCORES  # 32 local batches per core
G4 = BL // 4      # groups of 4 batches

# dtype knobs (accuracy/perf tradeoff)
H_NAT_BF16 = os.environ.get("KERN_H_BF16", "1") == "1"   # natural h (c_t path)
HT_BF16 = os.environ.get("KERN_HT_BF16", "1") == "1"     # transposed h (scores path)
RE1_BF16 = os.environ.get("KERN_RE1_BF16", "1") == "1"   # tanh output (scores matmul moving)

_cache = {}
DEBUG_GROUPS = int(os.environ.get("KERN_GROUPS", str(G4)))
SKIP_TAIL = os.environ.get("KERN_SKIP_TAIL", "0") == "1"
HP_BUFS = int(os.environ.get("KERN_HP_BUFS", "8"))


def _build_nc():
    from concourse import bacc, bass, mybir
    from concourse.tile import TileContext

    f32 = mybir.dt.float32
    bf16 = mybir.dt.bfloat16
    ht_dt = bf16 if HT_BF16 else f32
    h_dt = bf16 if H_NAT_BF16 else f32
    re1_dt = bf16 if RE1_BF16 else f32
    vd_dt = re1_dt  # scores-matmul stationary must not be lone-fp32
    AF = mybir.ActivationFunctionType

    nc = bacc.Bacc()

    htd_d = nc.declare_dram_parameter("htd", [BL // 2, 128, 2 * 2 * T], ht_dt, isOutput=False)
    hnd_d = nc.declare_dram_parameter("hnd", [BL // 2, 128, 2 * 4 * HE1], h_dt, isOutput=False)
    udt_d = nc.declare_dram_parameter("udt", [HE, HE], ht_dt, isOutput=False)
    vd2_d = nc.declare_dram_parameter("vd2", [128, 2], vd_dt, isOutput=False)
    wdb2_d = nc.declare_dram_parameter("wdb2", [128, 2], f32, isOutput=False)
    # gates-matmul moving blob: rows (W_ih col, b_ih+b_hh, W_ih col again);
    # with stationary rows (yt_attn, 1, c2) this folds both bias adds into PE
    wb3_d = nc.declare_dram_parameter("wb3", [3, 4 * HD], f32, isOutput=False)
    oc2_d = nc.declare_dram_parameter("oc2", [2, BL], f32, isOutput=False)
    ident_d = nc.declare_dram_parameter("ident", [128, 128], bf16, isOutput=False)
    outd_d = nc.declare_dram_parameter("out_d", [BL, HD], f32, isOutput=True)
    outc_d = nc.declare_dram_parameter("out_c", [BL, HE], f32, isOutput=True)

    with TileContext(nc) as tc:
        with (
            tc.tile_pool(name="const", bufs=1) as constp,
            tc.tile_pool(name="hp", bufs=HP_BUFS) as hp,
            tc.tile_pool(name="hnp", bufs=HP_BUFS) as hnp,
            tc.tile_pool(name="re1p", bufs=3) as re1p,
            tc.tile_pool(name="smp", bufs=3) as smp,
            # tiles read by a DMA (gather/out): long-recycle pool, so engine
            # ops rarely carry a WAR wait on a DMA semaphore
            tc.tile_pool(name="nrp", bufs=G4) as nrp,
            tc.tile_pool(name="tailp", bufs=1) as tailp,
            tc.tile_pool(name="tlp", bufs=2) as tlp,
            tc.tile_pool(name="udps", bufs=2, space="PSUM") as udps,
            tc.tile_pool(name="scps", bufs=1, space="PSUM") as scps,
            tc.tile_pool(name="trps", bufs=1, space="PSUM") as trps,
            tc.tile_pool(name="ctps", bufs=2, space="PSUM") as ctps,
        ):
            # ---- constants ----
            udt_sb = []
            for i in range(2):
                t_ = constp.tile([128, HE], ht_dt, tag=f"udt{i}")
                nc.sync.dma_start(out=t_[:], in_=udt_d[i * 128 : (i + 1) * 128, :])
                udt_sb.append(t_)
            vd2_sb = constp.tile([128, 2], vd_dt, tag="vd2")
            nc.sync.dma_start(out=vd2_sb[:], in_=vd2_d[:])
            wdb2_sb = constp.tile([128, 2], f32, tag="wdb2")
            nc.sync.dma_start(out=wdb2_sb[:], in_=wdb2_d[:])
            wb3_sb = constp.tile([3, 4 * HD], f32, tag="wb3")
            nc.sync.dma_start(out=wb3_sb[:], in_=wb3_d[:])
            # yt3: gates-matmul stationary; rows 1:3 (ones, c2) are constants,
            # row 0 is filled per tail block by the y_tilde gather DMA
            yt3 = tailp.tile([3, BL], f32, tag="yt3")
            nc.sync.dma_start(out=yt3[1:3, :], in_=oc2_d[:])
            ident_sb = constp.tile([128, 128], bf16, tag="ident")
            nc.sync.dma_start(out=ident_sb[:], in_=ident_d[:])

            # ---- per-engine warmups ----
            # Every engine observes each constant's DMA semaphore via a cheap
            # op up front; later ops then need <=1 sync wait (the hardware
            # instruction structs encode only one wait command).
            warm = ctps.tile([128, T], f32, tag="ct")
            for k, cst in enumerate(
                [udt_sb[0], udt_sb[1], vd2_sb, ident_sb, wb3_sb]
            ):
                nc.tensor.matmul(
                    warm[0:1, k : k + 1], cst[0:1, 0:1], cst[0:1, 0:1],
                    start=True, stop=True,
                )
            dscr = tailp.tile([1, 8], f32, tag="dscr")
            nc.vector.tensor_copy(dscr[0:1, 0:1], wb3_sb[0:1, 0:1])
            ascr = tailp.tile([1, 8], f32, tag="ascr")
            nc.scalar.copy(ascr[0:1, 0:1], wdb2_sb[0:1, 0:1])
            # chained: materialize the float-bias const AP on ACT only
            nc.scalar.activation(ascr[0:1, 1:2], ascr[0:1, 0:1], AF.Tanh, bias=0.0)

            ytacc = tailp.tile([128, G4], f32, tag="ytacc")

            def emit_tail(k):
                # ---- tail block for groups 2k,2k+1: gates + LSTM cell ----
                # gates = yt_attn*W_ih + 1*(b_ih+b_hh) + c2*W_ih via the K=3
                # stationary (yt3) against the wb3 moving blob; f gate unused
                # since c0 = 0. Emitted two groups after its gather so the PE
                # stream never stalls on the evac->gather chain.
                gps = []
                for half in range(2):
                    g_ = ctps.tile([128, T], f32, tag="ct")
                    absorb(g_)
                    nc.tensor.matmul(
                        g_[0:8, 0:512],
                        yt3[0:3, 8 * k : 8 * k + 8],
                        wb3_sb[:, half * 512 : (half + 1) * 512],
                    )
                    gps.append(g_)
                # gates: i=[0:256], g=[512:768], o=[768:1024] read from PSUM.
                # sigmoid(x) = (tanh(x/2)+1)/2 keeps the resident ACT table
                # (TANH+EXP) -- a SIGMOID op would trigger a 1.3us table load
                mul = mybir.AluOpType.mult
                add = mybir.AluOpType.add
                ti = tlp.tile([8, HD], f32, tag="ti")
                nc.scalar.activation(ti[:], gps[0][0:8, 0:256], AF.Tanh, scale=0.5)
                gg = tlp.tile([8, HD], f32, tag="gg")
                nc.scalar.activation(gg[:], gps[1][0:8, 0:256], AF.Tanh)
                to = tlp.tile([8, HD], f32, tag="to")
                nc.scalar.activation(to[:], gps[1][0:8, 256:512], AF.Tanh, scale=0.5)
                gi = tlp.tile([8, HD], f32, tag="gi")
                nc.vector.tensor_scalar(gi[:], ti[:], 1.0, 0.5, add, mul)
                go = tlp.tile([8, HD], f32, tag="go")
                nc.vector.tensor_scalar(go[:], to[:], 1.0, 0.5, add, mul)
                cnew = tlp.tile([8, HD], f32, tag="cnew")
                nc.vector.tensor_mul(cnew[:], gi[:], gg[:])
                tcn = tlp.tile([8, HD], f32, tag="tcn")
                nc.scalar.activation(tcn[:], cnew[:], AF.Tanh)
                dnew = nrp.tile([8, HD], f32, tag="dnew")
                nc.vector.tensor_mul(dnew[:], go[:], tcn[:])
                # dnew partition p = j*2+gg is batch 8k + 4*gg + j
                nc.sync.dma_start(
                    out=outd_d[8 * k : 8 * k + 8].rearrange(
                        "(gg j) e -> j gg e", j=4
                    ),
                    in_=dnew[:],
                )

            def absorb(ps, mov=None):
                # tiny const matmul into a freshly allocated PSUM tile: takes
                # over the slot-release wait so the first real matmul into the
                # tile carries only its own (single) cross-engine wait
                nc.tensor.matmul(
                    ps[0:1, 0:1], vd2_sb[0:1, 0:1],
                    (mov if mov is not None else vd2_sb)[0:1, 0:1],
                    start=True, stop=True,
                )

            # ---- per-group-of-4 pipeline ----
            tails_emitted = 0
            for g in range(DEBUG_GROUPS):
                # deferred tail blocks: block k's gather was issued at the end
                # of group 2k+1; emit its compute at the top of group 2k+3
                if not SKIP_TAIL and g % 2 == 1 and g >= 3:
                    emit_tail(tails_emitted)
                    tails_emitted += 1
                h_pair = [None, None]
                sc = scps.tile([128, T], f32, tag="sc")
                absorb(sc)
                for qq in range(2):  # two pairs of batches in this group
                    q = 2 * g + qq
                    # load one pair (2 batches): hT first (ud needs it first)
                    hT_sb = hp.tile([128, 2 * 2 * T], ht_dt, tag="ht")
                    nc.sync.dma_start(out=hT_sb[:], in_=htd_d[q])
                    h_sb = hnp.tile([128, 2 * 4 * HE1], h_dt, tag="hn")
                    nc.sync.dma_start(out=h_sb[:], in_=hnd_d[q])
                    h_pair[qq] = h_sb
                    # ud matmuls for both batches of the pair (PSUM-bank limit:
                    # a matmul output may not exceed 512 f32 columns)
                    ud_eo = []
                    for eo in range(2):
                        ud = udps.tile([128, 2 * T], f32, tag="ud")
                        absorb(ud)
                        ud_eo.append(ud)
                        for r in range(2):
                            for ei in range(2):
                                nc.tensor.matmul(
                                    ud[:, r * T : (r + 1) * T],
                                    udt_sb[ei][:, eo * 128 : (eo + 1) * 128],
                                    hT_sb[:, (ei * 2 + r) * T : (ei * 2 + r + 1) * T],
                                    start=(ei == 0),
                                    stop=(ei == 1),
                                )
                    # tanh(+bias) over the pair -> re1 [128, (eo, pair-batch, t)]
                    re1 = re1p.tile([128, 2 * 2 * T], re1_dt, tag="re1")
                    for eo in range(2):
                        nc.scalar.activation(
                            re1[:, eo * 2 * T : (eo + 1) * 2 * T],
                            ud_eo[eo][:],
                            AF.Tanh,
                            bias=wdb2_sb[:, eo : eo + 1],
                        )
                    # scores for both batches -> psum rows {0,32,64,96}
                    for r in range(2):
                        jj = 2 * qq + r
                        for ei in range(2):
                            nc.tensor.matmul(
                                sc[32 * jj : 32 * jj + 32, :],
                                vd2_sb[:, ei : ei + 1].broadcast_to([128, 32]),
                                re1[:, (ei * 2 + r) * T : (ei * 2 + r + 1) * T],
                                start=(ei == 0),
                                stop=(ei == 1),
                                tile_position=(0, 32 * jj),
                            )

                # ---- softmax over this group of 4, read directly from PSUM --
                # score rows live at partitions {0,32,64,96}; every partition p
                # holds a copy of batch p//32's scores, so full-128 ops work.
                mx = smp.tile([128, 1], f32, tag="mx")
                nc.vector.reduce_max(mx[:], sc[:], axis=mybir.AxisListType.X)
                nmx = smp.tile([128, 1], f32, tag="nmx")
                nc.vector.tensor_scalar_mul(nmx[:], mx[:], -1.0)
                pexp = nrp.tile([128, T], re1_dt, tag="pexp")
                zsum = smp.tile([128, 1], f32, tag="zsum")
                nc.scalar.activation(
                    pexp[:], sc[:], AF.Exp, bias=nmx[:], accum_out=zsum[:]
                )
                rz = smp.tile([128, 1], f32, tag="rz")
                nc.vector.reciprocal(rz[:], zsum[:])
                # gather the 4 (unnormalized) weight rows into contiguous
                # partitions (DMA moves freely across partitions; engines can't)
                beta = smp.tile([4, T], re1_dt, tag="beta")
                # DVE dummy write absorbs the slot-release wait so the gather
                # DMA carries only its producer wait
                nc.vector.tensor_copy(beta[0:1, 0:1], dscr[0:1, 0:1])
                nc.sync.dma_start(
                    out=beta[:],
                    in_=pexp[:].rearrange("(j s) t -> j s t", s=32)[:, 0, :],
                )

                # ---- transpose weights -> betaT columns [t=128 x 4tt, batch] --
                btr = trps.tile([128, 32], re1_dt, tag="btr")
                nc.tensor.transpose(
                    btr[0:1, 0:1], ident_sb[0:1, 0:1], ident_sb[0:1, 0:1]
                )
                for tt in range(4):
                    nc.tensor.transpose(
                        btr[:, tt * 4 : (tt + 1) * 4],
                        beta[:, tt * 128 : (tt + 1) * 128],
                        ident_sb[0:4, 0:4],
                    )
                betaT = smp.tile([128, 16], h_dt, tag="betaT")
                nc.vector.tensor_copy(betaT[:], btr[:, 0:16])

                # ---- c_t (+ folded wt column) for the 4 batches ----
                ct = ctps.tile([128, T], f32, tag="ct")
                absorb(ct, mov=h_pair[0])
                absorb(ct, mov=h_pair[1])
                for jj in range(4):
                    qq, r = divmod(jj, 2)
                    for tt in range(4):
                        nc.tensor.matmul(
                            ct[32 * jj : 32 * jj + 32, 0:HE1],
                            betaT[:, tt * 4 + jj : tt * 4 + jj + 1].broadcast_to(
                                [128, 32]
                            ),
                            h_pair[qq][:, (r * 4 + tt) * HE1 : (r * 4 + tt + 1) * HE1],
                            start=(tt == 0),
                            stop=(tt == 3),
                            tile_position=(0, 32 * jj),
                        )
                # single fused evacuation: normalize by 1/zsum on the way out
                ctstage = nrp.tile([128, HE1], f32, tag="ctstage")
                nc.vector.tensor_scalar_mul(ctstage[:], ct[:, 0:HE1], rz[:])
                # c_t output rows for this group (DMA un-strides the rows)
                nc.sync.dma_start(
                    out=outc_d[4 * g : 4 * g + 4, :],
                    in_=ctstage[:].rearrange("(j s) e -> j s e", s=32)[:, 0, 0:HE],
                )
                # wt . c_t partial of y_tilde came out in the folded column
                nc.vector.tensor_copy(ytacc[:, g : g + 1], ctstage[:, HE:HE1])

                # issue the y_tilde gather for this block of 2 groups; the
                # dependent compute is emitted two groups later (see top of
                # loop) so the gather latency is fully hidden
                if SKIP_TAIL or (g % 2 == 0):
                    continue
                k = g // 2
                # y_tilde-attn entries for groups 2k,2k+1 -> yt3 row 0, cols
                # [8k:8k+8]; stream order p = j*2+gg holds batch 8k+4*gg+j
                nc.sync.dma_start(
                    out=yt3[0:1, 8 * k : 8 * k + 8],
                    in_=ytacc[:].rearrange("(j s) g -> j s g", s=32)[
                        :, 0, 2 * k : 2 * k + 2
                    ],
                )

            if not SKIP_TAIL:
                for k in range(tails_emitted, DEBUG_GROUPS // 2):
                    emit_tail(k)

    nc.compile()
    return nc


def _prep_in_maps(inputs):
    h = np.asarray(inputs["h_t_enc"], np.float32)
    y = np.asarray(inputs["y"], np.float32)
    Ud_w = np.asarray(inputs["Ud_w"], np.float32)
    Ud_b = np.asarray(inputs["Ud_b"], np.float32)
    Wd_b = np.asarray(inputs["Wd_b"], np.float32)
    vd_w = np.asarray(inputs["vd_w"], np.float32)
    wt_w = np.asarray(inputs["wt_w"], np.float32)
    wt_b = np.asarray(inputs["wt_b"], np.float32)
    W_ih = np.asarray(inputs["W_ih"], np.float32)
    b_ih = np.asarray(inputs["b_ih"], np.float32)
    b_hh = np.asarray(inputs["b_hh"], np.float32)

    from ml_dtypes import bfloat16

    h_dt = bfloat16 if H_NAT_BF16 else np.float32
    ht_dt = bfloat16 if HT_BF16 else np.float32
    vd_dt = bfloat16 if RE1_BF16 else np.float32

    udt = np.ascontiguousarray(Ud_w.T).astype(ht_dt)
    vd2 = np.ascontiguousarray(vd_w[0].reshape(2, 128).T).astype(vd_dt)
    wdb2 = np.ascontiguousarray((Wd_b + Ud_b).reshape(2, 128).T)
    wb3 = np.ascontiguousarray(
        np.stack([W_ih[:, 0], b_ih + b_hh, W_ih[:, 0]], axis=0)
    )
    ident = np.eye(128, dtype=np.float32).astype(bfloat16)

    wt_vec = wt_w[0, :HE]

    def make_ht(hc):
        # hT region, ei-major: [pair, p, et(2), rb(2), t(512)]
        return np.ascontiguousarray(
            hc.transpose(0, 2, 1).reshape(BL // 2, 2, 2, 128, T)
            .transpose(0, 3, 2, 1, 4).reshape(BL // 2, 128, 2 * 2 * T)
        ).astype(ht_dt)

    def make_hn(hc):
        # natural rows + folded wt col: [pair, p, rb(2), tt(4), e(257)]
        hw = hc @ wt_vec  # [BL, T]
        nat = np.empty((BL // 2, 2, 4, 128, HE1), np.float32)
        nat[..., :HE] = hc.reshape(BL // 2, 2, 4, 128, HE)
        nat[..., HE] = hw.reshape(BL // 2, 2, 4, 128)
        return np.ascontiguousarray(
            nat.transpose(0, 3, 1, 2, 4).reshape(BL // 2, 128, 2 * 4 * HE1)
        ).astype(h_dt)

    in_maps = []
    for c in range(NCORES):
        sl = slice(c * BL, (c + 1) * BL)
        hc = h[sl]
        # per-batch constant part of y_tilde, in tail-block stream order:
        # oc2[1, 8k + 2j + gg] = c2v[8k + 4gg + j]; row 0 is the ones row
        c2v = wt_w[0, HE] * y[sl, 0, 0] + wt_b[0]  # [BL]
        oc2 = np.ones((2, BL), np.float32)
        for k in range(BL // 8):
            for j in range(4):
                for gg in range(2):
                    oc2[1, 8 * k + 2 * j + gg] = c2v[8 * k + 4 * gg + j]
        in_maps.append(
            {
                "htd": make_ht(hc),
                "hnd": make_hn(hc),
                "udt": udt,
                "vd2": vd2,
                "wdb2": wdb2,
                "wb3": wb3,
                "oc2": oc2,
                "ident": ident,
            }
        )
    return in_maps


def kernel(**inputs):
    from concourse.bass_utils import run_bass_kernel_spmd

    key = (H_NAT_BF16, HT_BF16, RE1_BF16)
    if key not in _cache:
        _cache[key] = _build_nc()
    nc = _cache[key]

    in_maps = _prep_in_maps(inputs)
    res = run_bass_kernel_spmd(nc, in_maps, list(range(NCORES)))
    kernel.last_results = res

    d_new = np.concatenate([np.asarray(r["out_d"]) for r in res.results], axis=0)
    c_t = np.concatenate([np.asarray(r["out_c"]) for r in res.results], axis=0)
    return np.stack([d_new.astype(np.float32), c_t.astype(np.float32)], axis=0)


kernel.last_results = None


# revision 32
# speedup vs baseline: 1.2806x; 1.0540x over previous
"""Trainium2 Bass kernel for DecoderWithTemporalAttention (single-step decode).

Math (reference collapses to, since initial decoder state is zero):
    re1    = tanh(h @ Ud_w.T + (Ud_b + Wd_b))          # [B, T, E]
    scores = re1 @ vd_w[0]                              # [B, T]  (+vd_b, dropped: softmax-invariant)
    beta   = softmax(scores, axis=T)
    c_t    = einsum('bt,bte->be', beta, h)              # [B, E]
    y_til  = concat([c_t, y[:,0]], -1) @ wt_w[0] + wt_b # [B]
    gates  = outer(y_til, W_ih[:,0]) + (b_ih + b_hh)    # [B, 4H]
    i,f,g,o = split(gates); d_new = sigmoid(o) * tanh(sigmoid(i) * tanh(g))
    returns stack([d_new, c_t])                         # [2, B, 256]

Sharding: pure data-parallel, batch 256 -> 8 cores x 32.

Device layout per core (BL=32 local batches):
  - hT  [BL,256,512] transposed on host, ei-major (moving operand for ud)
  - hn  [BL,512,257] natural rows + a 257th col h.wt (moving operand for c_t;
        the extra col folds the wt dot product into the same matmul)
  - ud matmul: stationary Ud_w.T tiles, out [e_out(128x2), (r,t)=1024] in PSUM
  - tanh+bias on ScalarE (per-partition bias, e_out on partitions) -> re1 bf16
  - scores: M=1 matmul vd-stationary -> PSUM [1,512] rows packed 4/bank at
    partitions {0,32,64,96}
  - softmax per group of 4 read directly from PSUM (max / exp+accum_out bf16)
  - unnormalized exp weights transposed via PE -> ct matmul; normalization is
    a single fused tensor_scalar_mul (x 1/zsum) at PSUM evacuation
  - LSTM-cell tail: K=1 outer-product matmul for gates, ACT sigmoid/tanh, DVE muls
"""

import os
import sys

import numpy as np

for _p in ("/opt/trn_rl_repo",):
    if _p not in sys.path and os.path.isdir(_p):
        sys.path.append(_p)

B, T, HE, HD = 256, 512, 256, 256
HE1 = HE + 1  # natural h row + folded wt column
NCORES = 8
BL = B // NCORES  # 32 local batches per core
G4 = BL // 4      # groups of 4 batches

# dtype knobs (accuracy/perf tradeoff)
H_NAT_BF16 = os.environ.get("KERN_H_BF16", "1") == "1"   # natural h (c_t path)
HT_BF16 = os.environ.get("KERN_HT_BF16", "1") == "1"     # transposed h (scores path)
RE1_BF16 = os.environ.get("KERN_RE1_BF16", "1") == "1"   # tanh output (scores matmul moving)

_cache = {}
DEBUG_GROUPS = int(os.environ.get("KERN_GROUPS", str(G4)))
SKIP_TAIL = os.environ.get("KERN_SKIP_TAIL", "0") == "1"
HP_BUFS = int(os.environ.get("KERN_HP_BUFS", "8"))


def _build_nc():
    from concourse import bacc, bass, mybir
    from concourse.tile import TileContext

    f32 = mybir.dt.float32
    bf16 = mybir.dt.bfloat16
    ht_dt = bf16 if HT_BF16 else f32
    h_dt = bf16 if H_NAT_BF16 else f32
    re1_dt = bf16 if RE1_BF16 else f32
    vd_dt = re1_dt  # scores-matmul stationary must not be lone-fp32
    AF = mybir.ActivationFunctionType

    nc = bacc.Bacc()

    # group-level h blobs: [group, partition, (pair, ...)-major free dim]
    htd_d = nc.declare_dram_parameter("htd", [G4, 128, 2 * 2 * 2 * T], ht_dt, isOutput=False)
    hnd_d = nc.declare_dram_parameter("hnd", [G4, 128, 2 * 2 * 4 * HE1], h_dt, isOutput=False)
    udt_d = nc.declare_dram_parameter("udt", [HE, HE], ht_dt, isOutput=False)
    # vd (cols 0:2) and the tanh bias Wd_b+Ud_b (cols 2:4), both bf16
    vw4_d = nc.declare_dram_parameter("vw4", [128, 4], vd_dt, isOutput=False)
    # gates-matmul moving blob: rows (W_ih col, b_ih+b_hh, W_ih col again);
    # with stationary rows (yt_attn, 1, c2) this folds both bias adds into PE.
    # cols: [0.5*i-gate (256) | g-gate (256) | 0.5*o-gate (256)] (f dropped)
    wb3_d = nc.declare_dram_parameter("wb3", [3, 3 * HD], vd_dt, isOutput=False)
    oc2_d = nc.declare_dram_parameter("oc2", [2, BL], vd_dt, isOutput=False)
    ident_d = nc.declare_dram_parameter("ident", [4, 4], bf16, isOutput=False)
    outd_d = nc.declare_dram_parameter("out_d", [BL, HD], f32, isOutput=True)
    outc_d = nc.declare_dram_parameter("out_c", [BL, HE], f32, isOutput=True)

    with TileContext(nc) as tc:
        with (
            tc.tile_pool(name="const", bufs=1) as constp,
            tc.tile_pool(name="hp", bufs=HP_BUFS) as hp,
            tc.tile_pool(name="hnp", bufs=HP_BUFS) as hnp,
            tc.tile_pool(name="re1p", bufs=3) as re1p,
            tc.tile_pool(name="smp", bufs=3) as smp,
            # tiles read by a DMA (gather/out): long-recycle pool, so engine
            # ops rarely carry a WAR wait on a DMA semaphore
            tc.tile_pool(name="nrp", bufs=G4) as nrp,
            tc.tile_pool(name="tailp", bufs=1) as tailp,
            tc.tile_pool(name="tlp", bufs=2) as tlp,
            tc.tile_pool(name="udps", bufs=2, space="PSUM") as udps,
            tc.tile_pool(name="scps", bufs=1, space="PSUM") as scps,
            tc.tile_pool(name="trps", bufs=1, space="PSUM") as trps,
            tc.tile_pool(name="ctps", bufs=2, space="PSUM") as ctps,
        ):
            # ---- group-0 h blob first: its first quarter gates the whole
            # pipeline, so issue it split in quarters (parallel queues) before
            # any other DMA; quarter order matches ud consumption order.
            hT0_sb = hp.tile([128, 2 * 2 * 2 * T], ht_dt, tag="ht")
            for s in (0, 2, 1, 3):
                nc.sync.dma_start(
                    out=hT0_sb[:, s * T : (s + 1) * T],
                    in_=htd_d[0, :, s * T : (s + 1) * T],
                )
            nc.sync.dma_start(
                out=hT0_sb[:, 4 * T : 8 * T], in_=htd_d[0, :, 4 * T : 8 * T]
            )
            udt_sb = []
            for i in range(2):
                t_ = constp.tile([128, HE], ht_dt, tag=f"udt{i}")
                nc.sync.dma_start(out=t_[:], in_=udt_d[i * 128 : (i + 1) * 128, :])
                udt_sb.append(t_)
            hn0_sb = hnp.tile([128, 2 * 2 * 4 * HE1], h_dt, tag="hn")
            nc.sync.dma_start(out=hn0_sb[:], in_=hnd_d[0])
            # ---- remaining constants ----
            vw4_sb = constp.tile([128, 4], vd_dt, tag="vw4")
            nc.sync.dma_start(out=vw4_sb[:], in_=vw4_d[:])
            vd2_sb = vw4_sb[:, 0:2]
            wb3_sb = constp.tile([3, 3 * HD], vd_dt, tag="wb3")
            nc.sync.dma_start(out=wb3_sb[:], in_=wb3_d[:])
            # yt3: gates-matmul stationary; rows 1:3 (ones, c2) are constants,
            # row 0 is filled per tail block by the y_tilde gather DMA
            yt3 = tailp.tile([3, BL], vd_dt, tag="yt3")
            nc.sync.dma_start(out=yt3[1:3, :], in_=oc2_d[:])
            ident_sb = constp.tile([4, 4], bf16, tag="ident")
            nc.sync.dma_start(out=ident_sb[:], in_=ident_d[:])

            # ---- per-engine warmups ----
            # Every engine observes each constant's DMA semaphore via a cheap
            # op up front; later ops then need <=1 sync wait (the hardware
            # instruction structs encode only one wait command).
            warm = ctps.tile([128, T], f32, tag="ct")
            for k, cst in enumerate(
                [udt_sb[0], udt_sb[1], vw4_sb, ident_sb, wb3_sb]
            ):
                nc.tensor.matmul(
                    warm[0:1, k : k + 1], cst[0:1, 0:1], cst[0:1, 0:1],
                    start=True, stop=True,
                )
            dscr = tailp.tile([1, 8], f32, tag="dscr")
            nc.vector.tensor_copy(dscr[0:1, 0:1], wb3_sb[0:1, 0:1])
            ascr = tailp.tile([1, 8], f32, tag="ascr")
            nc.scalar.copy(ascr[0:1, 0:1], vw4_sb[0:1, 0:1])
            # chained: materialize the float-bias const AP on ACT only
            nc.scalar.activation(ascr[0:1, 1:2], ascr[0:1, 0:1], AF.Tanh, bias=0.0)

            ytacc = tailp.tile([128, G4], vd_dt, tag="ytacc")

            def emit_tail(k):
                # ---- tail block for groups 2k,2k+1: gates + LSTM cell ----
                # gates = yt_attn*W_ih + 1*(b_ih+b_hh) + c2*W_ih via the K=3
                # stationary (yt3) against the wb3 moving blob; f gate unused
                # since c0 = 0. Emitted two groups after its gather so the PE
                # stream never stalls on the evac->gather chain.
                # sigmoid(x) = (tanh(x/2)+1)/2 keeps the resident ACT table
                # (TANH+EXP) resident -- the x/2 is pre-folded into wb3's
                # i/o-gate columns on the host.
                g0 = ctps.tile([128, T], f32, tag="ct")
                absorb(g0)
                nc.tensor.matmul(
                    g0[0:8, 0:512], yt3[0:3, 8 * k : 8 * k + 8], wb3_sb[:, 0:512]
                )
                g1 = ctps.tile([128, T], f32, tag="ct")
                absorb(g1)
                nc.tensor.matmul(
                    g1[0:8, 0:256], yt3[0:3, 8 * k : 8 * k + 8], wb3_sb[:, 512:768]
                )
                mul = mybir.AluOpType.mult
                add = mybir.AluOpType.add
                t0 = tlp.tile([8, 2 * HD], f32, tag="t0")  # [tanh(i/2) | tanh(g)]
                nc.scalar.activation(t0[:], g0[0:8, 0:512], AF.Tanh)
                to = tlp.tile([8, HD], f32, tag="to")
                nc.scalar.activation(to[:], g1[0:8, 0:256], AF.Tanh)
                gi = tlp.tile([8, HD], f32, tag="gi")
                nc.vector.tensor_scalar(gi[:], t0[:, 0:HD], 1.0, 0.5, add, mul)
                go = tlp.tile([8, HD], f32, tag="go")
                nc.vector.tensor_scalar(go[:], to[:], 1.0, 0.5, add, mul)
                cnew = tlp.tile([8, HD], f32, tag="cnew")
                nc.vector.tensor_mul(cnew[:], gi[:], t0[:, HD : 2 * HD])
                tcn = tlp.tile([8, HD], f32, tag="tcn")
                nc.scalar.activation(tcn[:], cnew[:], AF.Tanh)
                dnew = nrp.tile([8, HD], f32, tag="dnew")
                nc.vector.tensor_mul(dnew[:], go[:], tcn[:])
                # dnew partition p = j*2+gg is batch 8k + 4*gg + j
                nc.sync.dma_start(
                    out=outd_d[8 * k : 8 * k + 8].rearrange(
                        "(gg j) e -> j gg e", j=4
                    ),
                    in_=dnew[:],
                )

            def absorb(ps, mov=None):
                # tiny const matmul into a freshly allocated PSUM tile: takes
                # over the slot-release wait so the first real matmul into the
                # tile carries only its own (single) cross-engine wait
                nc.tensor.matmul(
                    ps[0:1, 0:1], vd2_sb[0:1, 0:1],
                    (mov if mov is not None else vd2_sb)[0:1, 0:1],
                    start=True, stop=True,
                )

            # ---- per-group-of-4 pipeline ----
            tails_emitted = 0
            for g in range(DEBUG_GROUPS):
                # deferred tail blocks: block k's gather was issued at the end
                # of group 2k+1; emit its compute at the top of group 2k+3
                if not SKIP_TAIL and g % 2 == 1 and g >= 3:
                    emit_tail(tails_emitted)
                    tails_emitted += 1
                # load the group blob (2 pairs): hT first (ud needs it first);
                # group 0's tiles were allocated + DMA'd before the constants
                if g == 0:
                    hTg_sb, hng_sb = hT0_sb, hn0_sb
                else:
                    hTg_sb = hp.tile([128, 2 * 2 * 2 * T], ht_dt, tag="ht")
                    nc.sync.dma_start(out=hTg_sb[:], in_=htd_d[g])
                    hng_sb = hnp.tile([128, 2 * 2 * 4 * HE1], h_dt, tag="hn")
                    nc.sync.dma_start(out=hng_sb[:], in_=hnd_d[g])
                h_pair = [
                    hng_sb[:, 0 : 8 * HE1],
                    hng_sb[:, 8 * HE1 : 16 * HE1],
                ]
                sc = scps.tile([128, T], f32, tag="sc")
                absorb(sc)
                for qq in range(2):  # two pairs of batches in this group
                    hT_sb = hTg_sb[:, qq * 4 * T : (qq + 1) * 4 * T]
                    # ud matmuls for both batches of the pair (PSUM-bank limit:
                    # a matmul output may not exceed 512 f32 columns)
                    ud_eo = []
                    for eo in range(2):
                        ud = udps.tile([128, 2 * T], f32, tag="ud")
                        absorb(ud)
                        ud_eo.append(ud)
                        for r in range(2):
                            for ei in range(2):
                                nc.tensor.matmul(
                                    ud[:, r * T : (r + 1) * T],
                                    udt_sb[ei][:, eo * 128 : (eo + 1) * 128],
                                    hT_sb[:, (ei * 2 + r) * T : (ei * 2 + r + 1) * T],
                                    start=(ei == 0),
                                    stop=(ei == 1),
                                )
                    # tanh(+bias) over the pair -> re1 [128, (eo, pair-batch, t)]
                    re1 = re1p.tile([128, 2 * 2 * T], re1_dt, tag="re1")
                    for eo in range(2):
                        nc.scalar.activation(
                            re1[:, eo * 2 * T : (eo + 1) * 2 * T],
                            ud_eo[eo][:],
                            AF.Tanh,
                            bias=vw4_sb[:, 2 + eo : 3 + eo],
                        )
                    # scores for both batches -> psum rows {0,32,64,96}
                    for r in range(2):
                        jj = 2 * qq + r
                        for ei in range(2):
                            nc.tensor.matmul(
                                sc[32 * jj : 32 * jj + 32, :],
                                vd2_sb[:, ei : ei + 1].broadcast_to([128, 32]),
                                re1[:, (ei * 2 + r) * T : (ei * 2 + r + 1) * T],
                                start=(ei == 0),
                                stop=(ei == 1),
                                tile_position=(0, 32 * jj),
                            )

                # ---- softmax over this group of 4, read directly from PSUM --
                # score rows live at partitions {0,32,64,96}; every partition p
                # holds a copy of batch p//32's scores, so full-128 ops work.
                mx = smp.tile([128, 1], f32, tag="mx")
                nc.vector.reduce_max(mx[:], sc[:], axis=mybir.AxisListType.X)
                nmx = smp.tile([128, 1], f32, tag="nmx")
                nc.vector.tensor_scalar_mul(nmx[:], mx[:], -1.0)
                pexp = nrp.tile([128, T], re1_dt, tag="pexp")
                zsum = smp.tile([128, 1], f32, tag="zsum")
                nc.scalar.activation(
                    pexp[:], sc[:], AF.Exp, bias=nmx[:], accum_out=zsum[:]
                )
                rz = smp.tile([128, 1], f32, tag="rz")
                nc.vector.reciprocal(rz[:], zsum[:])
                # gather the 4 (unnormalized) weight rows into contiguous
                # partitions (DMA moves freely across partitions; engines can't)
                beta = smp.tile([4, T], re1_dt, tag="beta")
                # DVE dummy write absorbs the slot-release wait so the gather
                # DMA carries only its producer wait
                nc.vector.tensor_copy(beta[0:1, 0:1], dscr[0:1, 0:1])
                nc.sync.dma_start(
                    out=beta[:],
                    in_=pexp[:].rearrange("(j s) t -> j s t", s=32)[:, 0, :],
                )

                # ---- transpose weights -> betaT columns [t=128 x 4tt, batch] --
                btr = trps.tile([128, 32], re1_dt, tag="btr")
                nc.tensor.transpose(
                    btr[0:1, 0:1], ident_sb[0:1, 0:1], ident_sb[0:1, 0:1]
                )
                for tt in range(4):
                    nc.tensor.transpose(
                        btr[:, tt * 4 : (tt + 1) * 4],
                        beta[:, tt * 128 : (tt + 1) * 128],
                        ident_sb[0:4, 0:4],
                    )
                betaT = smp.tile([128, 16], h_dt, tag="betaT")
                nc.vector.tensor_copy(betaT[:], btr[:, 0:16])

                # ---- c_t (+ folded wt column) for the 4 batches ----
                ct = ctps.tile([128, T], f32, tag="ct")
                absorb(ct, mov=h_pair[0])
                absorb(ct, mov=h_pair[1])
                for jj in range(4):
                    qq, r = divmod(jj, 2)
                    for tt in range(4):
                        nc.tensor.matmul(
                            ct[32 * jj : 32 * jj + 32, 0:HE1],
                            betaT[:, tt * 4 + jj : tt * 4 + jj + 1].broadcast_to(
                                [128, 32]
                            ),
                            h_pair[qq][:, (r * 4 + tt) * HE1 : (r * 4 + tt + 1) * HE1],
                            start=(tt == 0),
                            stop=(tt == 3),
                            tile_position=(0, 32 * jj),
                        )
                # single fused evacuation: normalize by 1/zsum on the way out
                ctstage = nrp.tile([128, HE1], f32, tag="ctstage")
                nc.vector.tensor_scalar_mul(ctstage[:], ct[:, 0:HE1], rz[:])
                # c_t output rows for this group (DMA un-strides the rows)
                nc.sync.dma_start(
                    out=outc_d[4 * g : 4 * g + 4, :],
                    in_=ctstage[:].rearrange("(j s) e -> j s e", s=32)[:, 0, 0:HE],
                )
                # wt . c_t partial of y_tilde came out in the folded column
                nc.vector.tensor_copy(ytacc[:, g : g + 1], ctstage[:, HE:HE1])

                # issue the y_tilde gather for this block of 2 groups; the
                # dependent compute is emitted two groups later (see top of
                # loop) so the gather latency is fully hidden
                if SKIP_TAIL or (g % 2 == 0):
                    continue
                k = g // 2
                # y_tilde-attn entries for groups 2k,2k+1 -> yt3 row 0, cols
                # [8k:8k+8]; stream order p = j*2+gg holds batch 8k+4*gg+j
                nc.sync.dma_start(
                    out=yt3[0:1, 8 * k : 8 * k + 8],
                    in_=ytacc[:].rearrange("(j s) g -> j s g", s=32)[
                        :, 0, 2 * k : 2 * k + 2
                    ],
                )

            if not SKIP_TAIL:
                for k in range(tails_emitted, DEBUG_GROUPS // 2):
                    emit_tail(k)

    nc.compile()
    return nc


def _prep_in_maps(inputs):
    h = np.asarray(inputs["h_t_enc"], np.float32)
    y = np.asarray(inputs["y"], np.float32)
    Ud_w = np.asarray(inputs["Ud_w"], np.float32)
    Ud_b = np.asarray(inputs["Ud_b"], np.float32)
    Wd_b = np.asarray(inputs["Wd_b"], np.float32)
    vd_w = np.asarray(inputs["vd_w"], np.float32)
    wt_w = np.asarray(inputs["wt_w"], np.float32)
    wt_b = np.asarray(inputs["wt_b"], np.float32)
    W_ih = np.asarray(inputs["W_ih"], np.float32)
    b_ih = np.asarray(inputs["b_ih"], np.float32)
    b_hh = np.asarray(inputs["b_hh"], np.float32)

    from ml_dtypes import bfloat16

    h_dt = bfloat16 if H_NAT_BF16 else np.float32
    ht_dt = bfloat16 if HT_BF16 else np.float32
    vd_dt = bfloat16 if RE1_BF16 else np.float32

    udt = np.ascontiguousarray(Ud_w.T).astype(ht_dt)
    vw4 = np.empty((128, 4), np.float32)
    vw4[:, 0:2] = vd_w[0].reshape(2, 128).T
    vw4[:, 2:4] = (Wd_b + Ud_b).reshape(2, 128).T
    vw4 = vw4.astype(vd_dt)
    # gates moving blob [3, 768]: cols [0.5*i | g | 0.5*o], rows (Wih, b, Wih)
    wfull = np.stack([W_ih[:, 0], b_ih + b_hh, W_ih[:, 0]], axis=0)  # [3, 4H]
    wb3 = np.concatenate(
        [0.5 * wfull[:, 0:HD], wfull[:, 2 * HD : 3 * HD], 0.5 * wfull[:, 3 * HD :]],
        axis=1,
    ).astype(vd_dt)
    ident = np.eye(4, dtype=np.float32).astype(bfloat16)

    wt_vec = wt_w[0, :HE]

    def make_ht(hc):
        # hT region: [group, p, pair(2), et(2), rb(2), t(512)]
        return np.ascontiguousarray(
            hc.transpose(0, 2, 1).reshape(G4, 2, 2, 2, 128, T)
            .transpose(0, 4, 1, 3, 2, 5).reshape(G4, 128, 8 * T)
        ).astype(ht_dt)

    def make_hn(hc):
        # natural rows + folded wt col: [group, p, pair(2), rb(2), tt(4), e(257)]
        hw = hc @ wt_vec  # [BL, T]
        nat = np.empty((G4, 2, 2, 4, 128, HE1), np.float32)
        nat[..., :HE] = hc.reshape(G4, 2, 2, 4, 128, HE)
        nat[..., HE] = hw.reshape(G4, 2, 2, 4, 128)
        return np.ascontiguousarray(
            nat.transpose(0, 4, 1, 2, 3, 5).reshape(G4, 128, 2 * 8 * HE1)
        ).astype(h_dt)

    in_maps = []
    for c in range(NCORES):
        sl = slice(c * BL, (c + 1) * BL)
        hc = h[sl]
        # per-batch constant part of y_tilde, in tail-block stream order:
        # oc2[1, 8k + 2j + gg] = c2v[8k + 4gg + j]; row 0 is the ones row
        c2v = wt_w[0, HE] * y[sl, 0, 0] + wt_b[0]  # [BL]
        oc2 = np.ones((2, BL), np.float32)
        for k in range(BL // 8):
            for j in range(4):
                for gg in range(2):
                    oc2[1, 8 * k + 2 * j + gg] = c2v[8 * k + 4 * gg + j]
        in_maps.append(
            {
                "htd": make_ht(hc),
                "hnd": make_hn(hc),
                "udt": udt,
                "vw4": vw4,
                "wb3": wb3,
                "oc2": oc2.astype(vd_dt),
                "ident": ident,
            }
        )
    return in_maps


def kernel(**inputs):
    from concourse.bass_utils import run_bass_kernel_spmd

    key = (H_NAT_BF16, HT_BF16, RE1_BF16)
    if key not in _cache:
        _cache[key] = _build_nc()
    nc = _cache[key]

    in_maps = _prep_in_maps(inputs)
    res = run_bass_kernel_spmd(nc, in_maps, list(range(NCORES)))
    kernel.last_results = res

    d_new = np.concatenate([np.asarray(r["out_d"]) for r in res.results], axis=0)
    c_t = np.concatenate([np.asarray(r["out_c"]) for r in res.results], axis=0)
    return np.stack([d_new.astype(np.float32), c_t.astype(np.float32)], axis=0)


kernel.last_results = None


# revision 45
# speedup vs baseline: 1.4826x; 1.1578x over previous
"""Trainium2 Bass kernel for DecoderWithTemporalAttention (single-step decode).

Math (reference collapses to, since initial decoder state is zero):
    re1    = tanh(h @ Ud_w.T + (Ud_b + Wd_b))          # [B, T, E]
    scores = re1 @ vd_w[0]                              # [B, T]  (+vd_b, dropped: softmax-invariant)
    beta   = softmax(scores, axis=T)
    c_t    = einsum('bt,bte->be', beta, h)              # [B, E]
    y_til  = concat([c_t, y[:,0]], -1) @ wt_w[0] + wt_b # [B]
    gates  = outer(y_til, W_ih[:,0]) + (b_ih + b_hh)    # [B, 4H]
    i,f,g,o = split(gates); d_new = sigmoid(o) * tanh(sigmoid(i) * tanh(g))
    returns stack([d_new, c_t])                         # [2, B, 256]

Sharding: pure data-parallel, batch 256 -> 8 cores x 32.

Device layout per core (BL=32 local batches):
  - hT  [BL,256,512] transposed on host, ei-major (moving operand for ud)
  - hn  [BL,512,257] natural rows + a 257th col h.wt (moving operand for c_t;
        the extra col folds the wt dot product into the same matmul)
  - ud matmul: stationary Ud_w.T tiles, out [e_out(128x2), (r,t)=1024] in PSUM
  - tanh+bias on ScalarE (per-partition bias, e_out on partitions) -> re1 bf16
  - scores: M=1 matmul vd-stationary -> PSUM [1,512] rows packed 4/bank at
    partitions {0,32,64,96}
  - softmax per group of 4 read directly from PSUM (max / exp+accum_out bf16)
  - unnormalized exp weights transposed via PE -> ct matmul; normalization is
    a single fused tensor_scalar_mul (x 1/zsum) at PSUM evacuation
  - LSTM-cell tail: K=1 outer-product matmul for gates, ACT sigmoid/tanh, DVE muls
"""

import os
import sys

import numpy as np

for _p in ("/opt/trn_rl_repo",):
    if _p not in sys.path and os.path.isdir(_p):
        sys.path.append(_p)

B, T, HE, HD = 256, 512, 256, 256
HE1 = HE + 1  # natural h row + folded wt column
NCORES = 8
BL = B // NCORES  # 32 local batches per core
G4 = BL // 4      # groups of 4 batches

# dtype knobs (accuracy/perf tradeoff)
H_NAT_BF16 = os.environ.get("KERN_H_BF16", "1") == "1"   # natural h (c_t path)
HT_BF16 = os.environ.get("KERN_HT_BF16", "1") == "1"     # transposed h (scores path)
RE1_BF16 = os.environ.get("KERN_RE1_BF16", "1") == "1"   # tanh output (scores matmul moving)
UD_FP8 = os.environ.get("KERN_UD_FP8", "1") == "1"       # hT+Ud fp8 -> DoubleRow ud matmul
RE1_FP8 = os.environ.get("KERN_RE1_FP8", "1") == "1"     # re1+vd fp8 -> DoubleRow scores

_cache = {}
DEBUG_GROUPS = int(os.environ.get("KERN_GROUPS", str(G4)))
SKIP_TAIL = os.environ.get("KERN_SKIP_TAIL", "0") == "1"
HP_BUFS = int(os.environ.get("KERN_HP_BUFS", "8"))


def _build_nc():
    from concourse import bacc, bass, mybir
    from concourse.tile import TileContext

    f32 = mybir.dt.float32
    bf16 = mybir.dt.bfloat16
    fp8 = mybir.dt.float8e4
    DR = mybir.MatmulPerfMode.DoubleRow
    ht_dt = fp8 if UD_FP8 else (bf16 if HT_BF16 else f32)
    h_dt = bf16 if H_NAT_BF16 else f32
    re1_dt = fp8 if RE1_FP8 else (bf16 if RE1_BF16 else f32)
    pex_dt = bf16  # softmax weights stay bf16 (c_t accuracy)
    vd_dt = bf16
    AF = mybir.ActivationFunctionType

    nc = bacc.Bacc()

    # group-level h blobs: [group, partition, (pair, ...)-major free dim]
    htd_d = nc.declare_dram_parameter("htd", [G4, 128, 2 * 2 * 2 * T], ht_dt, isOutput=False)
    hnd_d = nc.declare_dram_parameter("hnd", [G4, 128, 2 * 2 * 4 * HE1], h_dt, isOutput=False)
    # Ud_w.T for the ud matmul; fp8 DoubleRow uses free layout [i(2),eo(2),m]
    udt_d = nc.declare_dram_parameter(
        "udt", ([128, 512] if UD_FP8 else [HE, HE]), ht_dt, isOutput=False
    )
    # vd (cols 0:2) and the tanh bias Wd_b+Ud_b (cols 2:4), both bf16
    vw4_d = nc.declare_dram_parameter("vw4", [128, 4], vd_dt, isOutput=False)
    if RE1_FP8:
        # vd pre-broadcast for DoubleRow scores: [k, i(2)*m(32)] fp8
        vd8_d = nc.declare_dram_parameter("vd8", [128, 64], fp8, isOutput=False)
    # gates-matmul moving blob: rows (W_ih col, b_ih+b_hh, W_ih col again);
    # with stationary rows (yt_attn, 1, c2) this folds both bias adds into PE.
    # cols: [0.5*i-gate (256) | g-gate (256) | 0.5*o-gate (256)] (f dropped)
    wb3_d = nc.declare_dram_parameter("wb3", [3, 3 * HD], vd_dt, isOutput=False)
    oc2_d = nc.declare_dram_parameter("oc2", [2, BL], vd_dt, isOutput=False)
    ident_d = nc.declare_dram_parameter("ident", [4, 4], bf16, isOutput=False)
    outd_d = nc.declare_dram_parameter("out_d", [BL, HD], f32, isOutput=True)
    outc_d = nc.declare_dram_parameter("out_c", [BL, HE], f32, isOutput=True)

    with TileContext(nc) as tc:
        with (
            tc.tile_pool(name="const", bufs=1) as constp,
            tc.tile_pool(name="hp", bufs=HP_BUFS) as hp,
            tc.tile_pool(name="hnp", bufs=HP_BUFS) as hnp,
            tc.tile_pool(name="re1p", bufs=3) as re1p,
            tc.tile_pool(name="smp", bufs=3) as smp,
            # tiles read by a DMA (gather/out): long-recycle pool, so engine
            # ops rarely carry a WAR wait on a DMA semaphore
            tc.tile_pool(name="nrp", bufs=G4) as nrp,
            tc.tile_pool(name="tailp", bufs=1) as tailp,
            tc.tile_pool(name="tlp", bufs=2) as tlp,
            tc.tile_pool(name="udps", bufs=2, space="PSUM") as udps,
            tc.tile_pool(name="scps", bufs=1, space="PSUM") as scps,
            tc.tile_pool(name="trps", bufs=1, space="PSUM") as trps,
            tc.tile_pool(name="ctps", bufs=2, space="PSUM") as ctps,
        ):
            # ---- group-0 h blob first: its first quarter gates the whole
            # pipeline, so issue it split in quarters (parallel queues) before
            # any other DMA; quarter order matches ud consumption order.
            hT0_sb = hp.tile([128, 2 * 2 * 2 * T], ht_dt, tag="ht")
            for s in (0, 2, 1, 3):
                nc.sync.dma_start(
                    out=hT0_sb[:, s * T : (s + 1) * T],
                    in_=htd_d[0, :, s * T : (s + 1) * T],
                )
            nc.sync.dma_start(
                out=hT0_sb[:, 4 * T : 8 * T], in_=htd_d[0, :, 4 * T : 8 * T]
            )
            if UD_FP8:
                udt8_sb = constp.tile([128, 512], fp8, tag="udt8")
                nc.sync.dma_start(out=udt8_sb[:], in_=udt_d[:])
                udt_warm = [udt8_sb]
            else:
                udt_sb = []
                for i in range(2):
                    t_ = constp.tile([128, HE], ht_dt, tag=f"udt{i}")
                    nc.sync.dma_start(
                        out=t_[:], in_=udt_d[i * 128 : (i + 1) * 128, :]
                    )
                    udt_sb.append(t_)
                udt_warm = udt_sb
            hn0_sb = hnp.tile([128, 2 * 2 * 4 * HE1], h_dt, tag="hn")
            nc.sync.dma_start(out=hn0_sb[:], in_=hnd_d[0])
            # ---- remaining constants ----
            vw4_sb = constp.tile([128, 4], vd_dt, tag="vw4")
            nc.sync.dma_start(out=vw4_sb[:], in_=vw4_d[:])
            vd2_sb = vw4_sb[:, 0:2]
            if RE1_FP8:
                vd8_sb = constp.tile([128, 64], fp8, tag="vd8")
                nc.sync.dma_start(out=vd8_sb[:], in_=vd8_d[:])
            wb3_sb = constp.tile([3, 3 * HD], vd_dt, tag="wb3")
            nc.sync.dma_start(out=wb3_sb[:], in_=wb3_d[:])
            # yt3: gates-matmul stationary; rows 1:3 (ones, c2) are constants,
            # row 0 is filled per tail block by the y_tilde gather DMA
            yt3 = tailp.tile([3, BL], vd_dt, tag="yt3")
            nc.sync.dma_start(out=yt3[1:3, :], in_=oc2_d[:])
            ident_sb = constp.tile([4, 4], bf16, tag="ident")
            nc.sync.dma_start(out=ident_sb[:], in_=ident_d[:])

            # ---- per-engine warmups ----
            # Every engine observes each constant's DMA semaphore via a cheap
            # op up front; later ops then need <=1 sync wait (the hardware
            # instruction structs encode only one wait command).
            warm = ctps.tile([128, T], f32, tag="ct")
            warm_list = udt_warm + [vw4_sb, ident_sb, wb3_sb]
            if RE1_FP8:
                warm_list.append(vd8_sb)
            for k, cst in enumerate(warm_list):
                nc.tensor.matmul(
                    warm[0:1, k : k + 1], cst[0:1, 0:1], cst[0:1, 0:1],
                    start=True, stop=True,
                )
            dscr = tailp.tile([1, 8], f32, tag="dscr")
            nc.vector.tensor_copy(dscr[0:1, 0:1], wb3_sb[0:1, 0:1])
            ascr = tailp.tile([1, 8], f32, tag="ascr")
            nc.scalar.copy(ascr[0:1, 0:1], vw4_sb[0:1, 0:1])
            # chained: materialize the float-bias const AP on ACT only
            nc.scalar.activation(ascr[0:1, 1:2], ascr[0:1, 0:1], AF.Tanh, bias=0.0)

            ytacc = tailp.tile([128, G4], vd_dt, tag="ytacc")

            def emit_tail(k):
                # ---- tail block for groups 2k,2k+1: gates + LSTM cell ----
                # gates = yt_attn*W_ih + 1*(b_ih+b_hh) + c2*W_ih via the K=3
                # stationary (yt3) against the wb3 moving blob; f gate unused
                # since c0 = 0. Emitted two groups after its gather so the PE
                # stream never stalls on the evac->gather chain.
                # sigmoid(x) = (tanh(x/2)+1)/2 keeps the resident ACT table
                # (TANH+EXP) resident -- the x/2 is pre-folded into wb3's
                # i/o-gate columns on the host.
                g0 = ctps.tile([128, T], f32, tag="ct")
                absorb(g0)
                nc.tensor.matmul(
                    g0[0:8, 0:512], yt3[0:3, 8 * k : 8 * k + 8], wb3_sb[:, 0:512]
                )
                g1 = ctps.tile([128, T], f32, tag="ct")
                absorb(g1)
                nc.tensor.matmul(
                    g1[0:8, 0:256], yt3[0:3, 8 * k : 8 * k + 8], wb3_sb[:, 512:768]
                )
                mul = mybir.AluOpType.mult
                add = mybir.AluOpType.add
                t0 = tlp.tile([8, 2 * HD], f32, tag="t0")  # [tanh(i/2) | tanh(g)]
                nc.scalar.activation(t0[:], g0[0:8, 0:512], AF.Tanh)
                to = tlp.tile([8, HD], f32, tag="to")
                nc.scalar.activation(to[:], g1[0:8, 0:256], AF.Tanh)
                gi = tlp.tile([8, HD], f32, tag="gi")
                nc.vector.tensor_scalar(gi[:], t0[:, 0:HD], 1.0, 0.5, add, mul)
                go = tlp.tile([8, HD], f32, tag="go")
                nc.vector.tensor_scalar(go[:], to[:], 1.0, 0.5, add, mul)
                cnew = tlp.tile([8, HD], f32, tag="cnew")
                nc.vector.tensor_mul(cnew[:], gi[:], t0[:, HD : 2 * HD])
                tcn = tlp.tile([8, HD], f32, tag="tcn")
                nc.scalar.activation(tcn[:], cnew[:], AF.Tanh)
                dnew = nrp.tile([8, HD], f32, tag="dnew")
                nc.vector.tensor_mul(dnew[:], go[:], tcn[:])
                # dnew partition p = j*2+gg is batch 8k + 4*gg + j
                nc.sync.dma_start(
                    out=outd_d[8 * k : 8 * k + 8].rearrange(
                        "(gg j) e -> j gg e", j=4
                    ),
                    in_=dnew[:],
                )

            def absorb(ps, mov=None):
                # tiny const matmul into a freshly allocated PSUM tile: takes
                # over the slot-release wait so the first real matmul into the
                # tile carries only its own (single) cross-engine wait
                nc.tensor.matmul(
                    ps[0:1, 0:1], vd2_sb[0:1, 0:1],
                    (mov if mov is not None else vd2_sb)[0:1, 0:1],
                    start=True, stop=True,
                )

            # ---- per-group-of-4 pipeline ----
            tails_emitted = 0
            for g in range(DEBUG_GROUPS):
                # deferred tail blocks: block k's gather was issued at the end
                # of group 2k+1; emit its compute at the top of group 2k+3
                if not SKIP_TAIL and g % 2 == 1 and g >= 3:
                    emit_tail(tails_emitted)
                    tails_emitted += 1
                # load the group blob (2 pairs): hT first (ud needs it first);
                # group 0's tiles were allocated + DMA'd before the constants
                if g == 0:
                    hTg_sb, hng_sb = hT0_sb, hn0_sb
                else:
                    hTg_sb = hp.tile([128, 2 * 2 * 2 * T], ht_dt, tag="ht")
                    nc.sync.dma_start(out=hTg_sb[:], in_=htd_d[g])
                    hng_sb = hnp.tile([128, 2 * 2 * 4 * HE1], h_dt, tag="hn")
                    nc.sync.dma_start(out=hng_sb[:], in_=hnd_d[g])
                h_pair = [
                    hng_sb[:, 0 : 8 * HE1],
                    hng_sb[:, 8 * HE1 : 16 * HE1],
                ]
                sc = scps.tile([128, T], f32, tag="sc")
                absorb(sc)
                for qq in range(2):  # two pairs of batches in this group
                    hT_sb = hTg_sb[:, qq * 4 * T : (qq + 1) * 4 * T]
                    # ud matmuls for both batches of the pair (PSUM-bank limit:
                    # a matmul output may not exceed 512 f32 columns)
                    ud_eo = []
                    for eo in range(2):
                        ud = udps.tile([128, 2 * T], f32, tag="ud")
                        absorb(ud)
                        ud_eo.append(ud)
                        for r in range(2):
                            if UD_FP8:
                                # DoubleRow: both 128-wide k-tiles (the ei
                                # blocks) feed one matmul at 0.5 cyc/row
                                nc.tensor.matmul(
                                    ud[:, r * T : (r + 1) * T],
                                    udt8_sb[:].rearrange(
                                        "k (i eo m) -> k eo i m", i=2, eo=2
                                    )[:, eo, :, :],
                                    hT_sb.rearrange(
                                        "k (i r t) -> k r i t", i=2, r=2
                                    )[:, r, :, :],
                                    start=True,
                                    stop=True,
                                    perf_mode=DR,
                                )
                                continue
                            for ei in range(2):
                                nc.tensor.matmul(
                                    ud[:, r * T : (r + 1) * T],
                                    udt_sb[ei][:, eo * 128 : (eo + 1) * 128],
                                    hT_sb[:, (ei * 2 + r) * T : (ei * 2 + r + 1) * T],
                                    start=(ei == 0),
                                    stop=(ei == 1),
                                )
                    # tanh(+bias) over the pair -> re1 [128, (eo, pair-batch, t)]
                    re1 = re1p.tile([128, 2 * 2 * T], re1_dt, tag="re1")
                    for eo in range(2):
                        nc.scalar.activation(
                            re1[:, eo * 2 * T : (eo + 1) * 2 * T],
                            ud_eo[eo][:],
                            AF.Tanh,
                            bias=vw4_sb[:, 2 + eo : 3 + eo],
                        )
                    # scores for both batches -> psum rows {0,32,64,96}
                    for r in range(2):
                        jj = 2 * qq + r
                        if RE1_FP8:
                            nc.tensor.matmul(
                                sc[32 * jj : 32 * jj + 32, :],
                                vd8_sb[:].rearrange("k (i m) -> k i m", i=2),
                                re1[:].rearrange(
                                    "k (i r t) -> k r i t", i=2, r=2
                                )[:, r, :, :],
                                start=True,
                                stop=True,
                                tile_position=(0, 32 * jj),
                                perf_mode=DR,
                            )
                            continue
                        for ei in range(2):
                            nc.tensor.matmul(
                                sc[32 * jj : 32 * jj + 32, :],
                                vd2_sb[:, ei : ei + 1].broadcast_to([128, 32]),
                                re1[:, (ei * 2 + r) * T : (ei * 2 + r + 1) * T],
                                start=(ei == 0),
                                stop=(ei == 1),
                                tile_position=(0, 32 * jj),
                            )

                # ---- softmax over this group of 4, read directly from PSUM --
                # score rows live at partitions {0,32,64,96}; every partition p
                # holds a copy of batch p//32's scores, so full-128 ops work.
                mx = smp.tile([128, 1], f32, tag="mx")
                nc.vector.reduce_max(mx[:], sc[:], axis=mybir.AxisListType.X)
                nmx = smp.tile([128, 1], f32, tag="nmx")
                nc.vector.tensor_scalar_mul(nmx[:], mx[:], -1.0)
                pexp = nrp.tile([128, T], pex_dt, tag="pexp")
                zsum = smp.tile([128, 1], f32, tag="zsum")
                nc.scalar.activation(
                    pexp[:], sc[:], AF.Exp, bias=nmx[:], accum_out=zsum[:]
                )
                rz = smp.tile([128, 1], f32, tag="rz")
                nc.vector.reciprocal(rz[:], zsum[:])
                # gather the 4 (unnormalized) weight rows into contiguous
                # partitions (DMA moves freely across partitions; engines can't)
                beta = smp.tile([4, T], pex_dt, tag="beta")
                # DVE dummy write absorbs the slot-release wait so the gather
                # DMA carries only its producer wait
                nc.vector.tensor_copy(beta[0:1, 0:1], dscr[0:1, 0:1])
                nc.sync.dma_start(
                    out=beta[:],
                    in_=pexp[:].rearrange("(j s) t -> j s t", s=32)[:, 0, :],
                )

                # ---- transpose weights -> betaT columns [t=128 x 4tt, batch] --
                btr = trps.tile([128, 32], pex_dt, tag="btr")
                nc.tensor.transpose(
                    btr[0:1, 0:1], ident_sb[0:1, 0:1], ident_sb[0:1, 0:1]
                )
                for tt in range(4):
                    nc.tensor.transpose(
                        btr[:, tt * 4 : (tt + 1) * 4],
                        beta[:, tt * 128 : (tt + 1) * 128],
                        ident_sb[0:4, 0:4],
                    )
                betaT = smp.tile([128, 16], h_dt, tag="betaT")
                nc.vector.tensor_copy(betaT[:], btr[:, 0:16])

                # ---- c_t (+ folded wt column) for the 4 batches ----
                ct = ctps.tile([128, T], f32, tag="ct")
                absorb(ct, mov=h_pair[0])
                absorb(ct, mov=h_pair[1])
                for jj in range(4):
                    qq, r = divmod(jj, 2)
                    for tt in range(4):
                        nc.tensor.matmul(
                            ct[32 * jj : 32 * jj + 32, 0:HE1],
                            betaT[:, tt * 4 + jj : tt * 4 + jj + 1].broadcast_to(
                                [128, 32]
                            ),
                            h_pair[qq][:, (r * 4 + tt) * HE1 : (r * 4 + tt + 1) * HE1],
                            start=(tt == 0),
                            stop=(tt == 3),
                            tile_position=(0, 32 * jj),
                        )
                # single fused evacuation: normalize by 1/zsum on the way out
                ctstage = nrp.tile([128, HE1], f32, tag="ctstage")
                nc.vector.tensor_scalar_mul(ctstage[:], ct[:, 0:HE1], rz[:])
                # c_t output rows for this group (DMA un-strides the rows)
                nc.sync.dma_start(
                    out=outc_d[4 * g : 4 * g + 4, :],
                    in_=ctstage[:].rearrange("(j s) e -> j s e", s=32)[:, 0, 0:HE],
                )
                # wt . c_t partial of y_tilde came out in the folded column
                nc.vector.tensor_copy(ytacc[:, g : g + 1], ctstage[:, HE:HE1])

                # issue the y_tilde gather for this block of 2 groups; the
                # dependent compute is emitted two groups later (see top of
                # loop) so the gather latency is fully hidden
                if SKIP_TAIL or (g % 2 == 0):
                    continue
                k = g // 2
                # y_tilde-attn entries for groups 2k,2k+1 -> yt3 row 0, cols
                # [8k:8k+8]; stream order p = j*2+gg holds batch 8k+4*gg+j
                nc.sync.dma_start(
                    out=yt3[0:1, 8 * k : 8 * k + 8],
                    in_=ytacc[:].rearrange("(j s) g -> j s g", s=32)[
                        :, 0, 2 * k : 2 * k + 2
                    ],
                )

            if not SKIP_TAIL:
                for k in range(tails_emitted, DEBUG_GROUPS // 2):
                    emit_tail(k)

    nc.compile()
    return nc


def _prep_in_maps(inputs):
    h = np.asarray(inputs["h_t_enc"], np.float32)
    y = np.asarray(inputs["y"], np.float32)
    Ud_w = np.asarray(inputs["Ud_w"], np.float32)
    Ud_b = np.asarray(inputs["Ud_b"], np.float32)
    Wd_b = np.asarray(inputs["Wd_b"], np.float32)
    vd_w = np.asarray(inputs["vd_w"], np.float32)
    wt_w = np.asarray(inputs["wt_w"], np.float32)
    wt_b = np.asarray(inputs["wt_b"], np.float32)
    W_ih = np.asarray(inputs["W_ih"], np.float32)
    b_ih = np.asarray(inputs["b_ih"], np.float32)
    b_hh = np.asarray(inputs["b_hh"], np.float32)

    from ml_dtypes import bfloat16, float8_e4m3fn

    h_dt = bfloat16 if H_NAT_BF16 else np.float32
    ht_dt = float8_e4m3fn if UD_FP8 else (bfloat16 if HT_BF16 else np.float32)
    vd_dt = bfloat16

    if UD_FP8:
        # [k, i(2), eo(2), m(128)] free layout for the DoubleRow stationary
        udt = np.ascontiguousarray(
            Ud_w.T.reshape(2, 128, 2, 128).transpose(1, 0, 2, 3).reshape(128, 512)
        ).astype(float8_e4m3fn)
    else:
        udt = np.ascontiguousarray(Ud_w.T).astype(ht_dt)
    vw4 = np.empty((128, 4), np.float32)
    vw4[:, 0:2] = vd_w[0].reshape(2, 128).T
    vw4[:, 2:4] = (Wd_b + Ud_b).reshape(2, 128).T
    vw4 = vw4.astype(vd_dt)
    # vd pre-broadcast for DoubleRow scores: vd8[k, i*32+m] = vd[i*128+k]
    vd8 = np.ascontiguousarray(
        np.repeat(vd_w[0].reshape(2, 128).T[:, :, None], 32, axis=2).reshape(128, 64)
    ).astype(float8_e4m3fn)
    # gates moving blob [3, 768]: cols [0.5*i | g | 0.5*o], rows (Wih, b, Wih)
    wfull = np.stack([W_ih[:, 0], b_ih + b_hh, W_ih[:, 0]], axis=0)  # [3, 4H]
    wb3 = np.concatenate(
        [0.5 * wfull[:, 0:HD], wfull[:, 2 * HD : 3 * HD], 0.5 * wfull[:, 3 * HD :]],
        axis=1,
    ).astype(vd_dt)
    ident = np.eye(4, dtype=np.float32).astype(bfloat16)

    wt_vec = wt_w[0, :HE]

    def make_ht(hc):
        # hT region: [group, p, pair(2), et(2), rb(2), t(512)]
        return np.ascontiguousarray(
            hc.transpose(0, 2, 1).reshape(G4, 2, 2, 2, 128, T)
            .transpose(0, 4, 1, 3, 2, 5).reshape(G4, 128, 8 * T)
        ).astype(ht_dt)

    def make_hn(hc):
        # natural rows + folded wt col: [group, p, pair(2), rb(2), tt(4), e(257)]
        hw = hc @ wt_vec  # [BL, T]
        nat = np.empty((G4, 2, 2, 4, 128, HE1), np.float32)
        nat[..., :HE] = hc.reshape(G4, 2, 2, 4, 128, HE)
        nat[..., HE] = hw.reshape(G4, 2, 2, 4, 128)
        return np.ascontiguousarray(
            nat.transpose(0, 4, 1, 2, 3, 5).reshape(G4, 128, 2 * 8 * HE1)
        ).astype(h_dt)

    in_maps = []
    for c in range(NCORES):
        sl = slice(c * BL, (c + 1) * BL)
        hc = h[sl]
        # per-batch constant part of y_tilde, in tail-block stream order:
        # oc2[1, 8k + 2j + gg] = c2v[8k + 4gg + j]; row 0 is the ones row
        c2v = wt_w[0, HE] * y[sl, 0, 0] + wt_b[0]  # [BL]
        oc2 = np.ones((2, BL), np.float32)
        for k in range(BL // 8):
            for j in range(4):
                for gg in range(2):
                    oc2[1, 8 * k + 2 * j + gg] = c2v[8 * k + 4 * gg + j]
        im = {
            "htd": make_ht(hc),
            "hnd": make_hn(hc),
            "udt": udt,
            "vw4": vw4,
            "wb3": wb3,
            "oc2": oc2.astype(vd_dt),
            "ident": ident,
        }
        if RE1_FP8:
            im["vd8"] = vd8
        in_maps.append(im)
    return in_maps


def kernel(**inputs):
    from concourse.bass_utils import run_bass_kernel_spmd

    key = (H_NAT_BF16, HT_BF16, RE1_BF16, UD_FP8, RE1_FP8)
    if key not in _cache:
        _cache[key] = _build_nc()
    nc = _cache[key]

    in_maps = _prep_in_maps(inputs)
    res = run_bass_kernel_spmd(nc, in_maps, list(range(NCORES)))
    kernel.last_results = res

    d_new = np.concatenate([np.asarray(r["out_d"]) for r in res.results], axis=0)
    c_t = np.concatenate([np.asarray(r["out_c"]) for r in res.results], axis=0)
    return np.stack([d_new.astype(np.float32), c_t.astype(np.float32)], axis=0)


kernel.last_results = None
